# Initial kernel scaffold
#
"""Trainium2 Bass kernel for channel-wise ("transposed") attention.

Reference computation (per batch b, X = x_in[b] reshaped [N=16384, C=256]):
    Q = X Wq ; K = X Wk ; V = X Wv            (columns l2-normalized over tokens for Q,K)
    attn[h,i,j] = softmax_j( khat_i . qhat_j * rescale[h] )   (32x32 per head)
    out = (A_bd @ V^T)^T Wp + bp

Algebraic reduction (validated vs reference):
    S    = X^T X                      [256,256]   (only pass-1 reduction needed)
    P1   = S Wq ; P2 = S Wk
    G    = Wk^T P1                    (raw cross-gram K^T Q)
    nq2  = diag(Wq^T P1) ; nk2 = diag(Wk^T P2)
    L    = G * rk[i] * (rq*rescale_expanded)[j] ;  A = blockdiag-softmax_j(exp(L))
    Wbig = Wv @ (A_bd^T Wp)           [256,256]
    out  = X @ Wbig + bp

Schedule (per core = one batch, data parallel, no collectives):
  pass 1   stream X f32 -> bf16 SBUF (casting DMA halves modeled DMA cost);
           X stays fully resident (64KB/partition).  PE does only the S
           accumulation here, so pass 1 is input-DMA + S-matmul bound.
  phase B  tiny 256x256 chains -> Wbig.  Single activation-table set
           (ln/exp/copy) loaded once at t=0: zero on-path table loads;
           rsqrt via exp(-0.5 ln x); rescale pre-folded into a scaled Wq
           copy used only by the norm fork (exact for rescale > 0).
  pass 2   per output group: transpose that group's X tiles on PE (bf16,
           1 cyc/row) -> evict to xT -> out = X @ Wbig + bp -> f32 DMA out.
           Output DMA (46.6us) is the bound; transposes live in PE slack.
"""

import sys

if "/opt/trn_rl_repo" not in sys.path:
    sys.path.insert(0, "/opt/trn_rl_repo")

from contextlib import ExitStack

import numpy as np

import concourse.bass as bass
import concourse.tile as tile
from concourse import bacc, mybir
from concourse import bass_utils
from concourse.bass import ds, ts
from concourse.bass_interp import get_hw_module
from concourse.masks import make_identity

F32 = mybir.dt.float32
F32R = mybir.dt.float32r
BF16 = mybir.dt.bfloat16
ALU = mybir.AluOpType
ACTF = mybir.ActivationFunctionType
PSUM = bass.MemorySpace.PSUM

N_CORES = 8
B, H, W, C = 8, 128, 128, 256
HEADS, DH = 8, 32
N = H * W            # 16384 tokens per batch
P = 128              # partitions / token tile
NT = N // P          # 128 token tiles
DMA_TILES = 8        # token tiles per DMA (1 MiB f32 chunks)
NG = NT // DMA_TILES # 16 groups
NCHUNK = C // P      # 2 channel chunks

# act_func_sets index of natural_log_exp_and_others: {ln, exp, copy, ...}
ACT_SET_LN_EXP = 6


def _build_kernel(nc: bacc.Bacc):
    x_dram = nc.dram_tensor("x_in", [N, C], F32, kind="ExternalInput").ap()
    wq_dram = nc.dram_tensor("Wq", [C, C], F32, kind="ExternalInput").ap()
    wk_dram = nc.dram_tensor("Wk", [C, C], F32, kind="ExternalInput").ap()
    wv_dram = nc.dram_tensor("Wv", [C, C], F32, kind="ExternalInput").ap()
    resc_dram = nc.dram_tensor("rescale", [HEADS, 1, 1], F32, kind="ExternalInput").ap()
    wp_dram = nc.dram_tensor("Wp", [C, C], F32, kind="ExternalInput").ap()
    bp_dram = nc.dram_tensor("bp", [C], F32, kind="ExternalInput").ap()
    out_dram = nc.dram_tensor("out", [N, C], F32, kind="ExternalOutput").ap()

    with tile.TileContext(nc) as tc, ExitStack() as top:
        consts = top.enter_context(tc.tile_pool(name="consts", bufs=1))
        xt_pool = top.enter_context(tc.tile_pool(name="xt", bufs=1))
        xf_pool = top.enter_context(tc.tile_pool(name="xfull", bufs=1))
        s_stack = ExitStack()
        s_pool = s_stack.enter_context(tc.tile_pool(name="spsum", bufs=1, space=PSUM))

        # ------------- const tiles (instructions emitted inside pass-1 g==0) -------------
        identity_f = consts.tile([P, P], F32)
        identity = consts.tile([P, P], BF16)     # bf16: 1 cyc/row transposes
        p8 = consts.tile([HEADS, C], F32)        # p8[h,c] = 1 iff c//32 == h
        p8_r = consts.tile([HEADS, C], F32R)
        bdmask = consts.tile([P, NCHUNK, C], F32)  # block-diag head mask chunks
        ones_col_f = consts.tile([P, 1], F32)
        ones_col = consts.tile([P, 1], F32R)     # [128,1] ones: column-sum matmuls
        ones_row = consts.tile([1, P], F32)      # [1,128] ones: partition broadcast
        ones_row_r = consts.tile([1, P], F32R)

        # weight tiles (DMAs issued after the x loads to keep x at queue head)
        wqk = consts.tile([P, NCHUNK, 2 * C], F32)       # [Wq | Wk] row chunks
        wp_sb = consts.tile([P, NCHUNK, C], F32)
        wv_sb = consts.tile([P, NCHUNK, C], F32)
        wvT = consts.tile([P, NCHUNK, C], F32R)          # wvT[p,k,c] = Wv[c, 128k+p]
        wqk_r = consts.tile([P, NCHUNK, 2 * C], F32R)    # rounded copies for f32r mms
        wp_r = consts.tile([P, NCHUNK, C], F32R)
        bp_sb = consts.tile([1, C], F32)
        bp2_r = consts.tile([1, 2 * C], F32R)    # [bp | bp] row for bias matmuls
        resc_p = consts.tile([HEADS, 1], F32)
        resc_r = consts.tile([HEADS, 1], F32R)
        rexp_row = consts.tile([1, C], F32)      # rescale broadcast over head blocks
        rexp1i = consts.tile([1, C], F32)        # rexp^-1 row
        rexp2i = consts.tile([1, C], F32)        # rexp^-2 row
        wq_scaled = consts.tile([P, NCHUNK, C], F32)  # Wq * rexp^-2 (qp/nq2 only)
        bias_bc = consts.tile([P, 2 * C], F32)   # [bp | bp] broadcast down partitions
        wbig0 = consts.tile([P, C], BF16)
        wbig1 = consts.tile([P, C], BF16)
        wbig_l = [wbig0, wbig1]

        # X^T (bf16), built in pass 2; one tensor per output group so the
        # out-matmuls of group g depend only on group g's transposes
        xg = [xf_pool.tile([P, DMA_TILES, C], BF16, name=f"xg{g}") for g in range(NG)]
        GROUPS = [2, 2, 2, 2] + [8] * 15
        assert sum(GROUPS) == NT
        starts = [sum(GROUPS[:i]) for i in range(len(GROUPS))]
        xTg = [
            xt_pool.tile([P, NCHUNK, gsz * P], BF16, name=f"xTg{gi}")
            for gi, gsz in enumerate(GROUPS)
        ]

        s_ps0 = s_pool.tile([P, C], F32, space=PSUM)
        s_ps1 = s_pool.tile([P, C], F32, space=PSUM)
        s_ps = [s_ps0, s_ps1]

        # ---------------- pass 1: load X (bf16), S = X^T X ----------------
        with tc.tile_pool(name="tp", bufs=4, space=PSUM) as tp_pool:
            for g in range(NG):
                if g == 0:
                    # small first piece so PE starts sooner
                    for lo, n_t in ((0, 4), (4, 4)):
                        nc.gpsimd.dma_start(
                            xg[g][:, ds(lo, n_t), :],
                            x_dram[ds((g * DMA_TILES + lo) * P, n_t * P), :].rearrange(
                                "(a p) c -> p a c", p=P
                            ),
                        )
                else:
                    nc.gpsimd.dma_start(
                        xg[g][:],
                        x_dram[ds(g * DMA_TILES * P, DMA_TILES * P), :].rearrange(
                            "(a p) c -> p a c", p=P
                        ),
                    )
                if g == 0:
                    # single activation-table load for the whole kernel
                    nc.scalar.add_instruction(
                        mybir.InstLoadActFuncSet(
                            name=nc.get_next_instruction_name(),
                            act_func_set_id=ACT_SET_LN_EXP,
                            ins=[],
                            outs=[],
                        )
                    )
                    make_identity(nc, identity_f[:])
                    nc.vector.tensor_copy(identity[:], identity_f[:])
                    nc.gpsimd.memset(p8[:], 0.0)
                    nc.gpsimd.affine_select(
                        out=p8[:].rearrange("p (b i) -> p b i", i=DH),
                        in_=p8[:].rearrange("p (b i) -> p b i", i=DH),
                        compare_op=ALU.not_equal,
                        fill=1.0,
                        base=0,
                        pattern=[[-1, HEADS], [0, DH]],
                        channel_multiplier=1,
                    )
                    nc.vector.tensor_copy(p8_r[:], p8[:])
                    nc.gpsimd.memset(bdmask[:], 0.0)
                    for r in range(NCHUNK):
                        for a2 in range(P // DH):
                            nc.gpsimd.memset(
                                bdmask[ts(a2, DH), r, ds(r * P + a2 * DH, DH)], 1.0
                            )
                    nc.gpsimd.memset(ones_col_f[:], 1.0)
                    nc.vector.tensor_copy(ones_col[:], ones_col_f[:])
                    nc.gpsimd.memset(ones_row[:], 1.0)
                    nc.vector.tensor_copy(ones_row_r[:], ones_row[:])
                if g == 1:
                    # weight/bias loads + prep: issued behind the first x chunk
                    for k in range(NCHUNK):
                        nc.sync.dma_start(wqk[:, k, 0:C], wq_dram[ts(k, P), :])
                        nc.sync.dma_start(wqk[:, k, C : 2 * C], wk_dram[ts(k, P), :])
                        nc.sync.dma_start(wp_sb[:, k, :], wp_dram[ts(k, P), :])
                        nc.sync.dma_start(wv_sb[:, k, :], wv_dram[ts(k, P), :])
                    nc.sync.dma_start(bp_sb[:], bp_dram.rearrange("(a c) -> a c", a=1))
                    nc.sync.dma_start(resc_p[:], resc_dram.rearrange("h a b -> h (a b)"))
                    for k in range(NCHUNK):
                        nc.vector.tensor_copy(wqk_r[:, k, :], wqk[:, k, :])
                        nc.vector.tensor_copy(wp_r[:, k, :], wp_sb[:, k, :])
                    nc.vector.tensor_copy(bp2_r[:, 0:C], bp_sb[:])
                    nc.vector.tensor_copy(bp2_r[:, C : 2 * C], bp_sb[:])
                    nc.vector.tensor_copy(resc_r[:], resc_p[:])
            # S accumulation.  The weight-prep matmuls are interleaved right
            # where PE would otherwise stall waiting for early DMA groups, so
            # PE ramps once and never resets pstate.
            def s_tile(t, first=False, last=False):
                g, a = divmod(t, DMA_TILES)
                x_t = xg[g][:, a, :]
                for k in range(NCHUNK):
                    nc.tensor.matmul(
                        s_ps[k][:],
                        x_t[:, ts(k, P)],
                        x_t[:],
                        start=first and k == 0,
                        stop=last and k == 1,
                    )

            s_tile(0, first=True)
            s_tile(1)
            s_tile(2)
            s_tile(3)
            # prep block 1: Wv transposes, rescale row, bias broadcast (PE)
            for k in range(NCHUNK):
                for m in range(NCHUNK):
                    tpv = tp_pool.tile([P, P], F32, space=PSUM, tag="tp")
                    nc.tensor.transpose(
                        tpv[:].bitcast(F32), wv_sb[:, m, ts(k, P)], identity_f[:]
                    )
                    nc.vector.tensor_copy(wvT[:, k, ts(m, P)], tpv[:].bitcast(F32))
            rexp_ps = tp_pool.tile([P, C], F32, space=PSUM, tag="tp")
            nc.tensor.matmul(
                rexp_ps[0:1, :], resc_r[:], p8_r[:], start=True, stop=True
            )
            nc.vector.tensor_copy(rexp_row[:], rexp_ps[0:1, :])
            nc.vector.reciprocal(rexp1i[:], rexp_row[:])
            nc.vector.tensor_mul(rexp2i[:], rexp1i[:], rexp1i[:])
            bb_ps = tp_pool.tile([P, 2 * C], F32, space=PSUM, tag="tp")
            nc.tensor.matmul(
                bb_ps[:], ones_row_r[:], bp2_r[:], start=True, stop=True
            )
            nc.scalar.copy(bias_bc[:], bb_ps[:])
            for t in range(4, 8):
                s_tile(t)
            for t in range(8, 16):
                s_tile(t)
            # prep block 2: rexp^-2 broadcast + scaled Wq (norm-fork input)
            r2bc_ps = tp_pool.tile([P, C], F32, space=PSUM, tag="tp")
            nc.tensor.matmul(
                r2bc_ps[:], ones_row[:], rexp2i[:], start=True, stop=True
            )
            for k in range(NCHUNK):
                nc.vector.tensor_mul(wq_scaled[:, k, :], wqk[:, k, 0:C], r2bc_ps[:])
            for t in range(16, NT):
                s_tile(t, last=(t == NT - 1))

        # ---------------- phase B: 256x256 attention math ----------------
        # Per-chunk tensors so chunk-0 consumers never wait on chunk-1 writes.
        # PSUM evictions alternate DVE/ACT; the softmax path (P1 -> G -> t ->
        # e -> A -> T1 -> Wbig) is kept separate from the norm forks, which
        # read PSUM directly and merge only at the Exp.
        with tc.tile_pool(name="bwork", bufs=4, space=PSUM) as bwork, tc.tile_pool(
            name="bsmall", bufs=2, space=PSUM
        ) as bsmall, tc.tile_pool(name="bsb", bufs=1) as bsb:
            s_sbl = []
            for k in range(NCHUNK):
                s_k = bsb.tile([P, C], F32R, name=f"s_sb{k}", tag="ssb", bufs=2)
                if k == 0:
                    nc.vector.tensor_copy(s_k[:], s_ps[k][:])
                else:
                    nc.scalar.copy(s_k[:], s_ps[k][:])
                s_sbl.append(s_k)

            # P1 = S @ Wq, P2 = S @ Wk   (uses S symmetric: lhsT = S chunks)
            p1_psl, p2_psl = [], []
            for m in range(NCHUNK):
                pp = bwork.tile([P, C], F32, space=PSUM, name=f"p1ps{m}", tag="bw", bufs=4)
                for k in range(NCHUNK):
                    nc.tensor.matmul(
                        pp[:],
                        s_sbl[k][:, ts(m, P)],
                        wqk_r[:, k, 0:C],
                        start=(k == 0),
                        stop=(k == 1),
                    )
                p1_psl.append(pp)
            for m in range(NCHUNK):
                pp = bwork.tile([P, C], F32, space=PSUM, name=f"p2ps{m}", tag="bw", bufs=4)
                for k in range(NCHUNK):
                    nc.tensor.matmul(
                        pp[:],
                        s_sbl[k][:, ts(m, P)],
                        wqk_r[:, k, C : 2 * C],
                        start=(k == 0),
                        stop=(k == 1),
                    )
                p2_psl.append(pp)
            p1_sbl = []
            for m in range(NCHUNK):
                psb = bsb.tile([P, C], F32R, name=f"p1sb{m}", tag="p1sb", bufs=2)
                if m == 0:
                    nc.vector.tensor_copy(psb[:], p1_psl[m][:])
                else:
                    nc.scalar.copy(psb[:], p1_psl[m][:])
                p1_sbl.append(psb)

            # norm fork #1: nq2*rexp^-2 via wq_scaled; rq = rsqrt -> rq*rescale
            qpl = []
            for m in range(NCHUNK):
                qp = bsb.tile([P, C], F32R, name=f"qp{m}", tag="qp", bufs=2)
                nc.vector.tensor_mul(qp[:], wq_scaled[:, m, :], p1_psl[m][:])
                qpl.append(qp)
            nq2_ps = bsmall.tile([1, C], F32, space=PSUM, tag="bs")
            for k in range(NCHUNK):
                nc.tensor.matmul(
                    nq2_ps[:], ones_col[:], qpl[k][:], start=(k == 0), stop=(k == 1)
                )
            lnq = bsb.tile([1, C], F32)
            nc.scalar.activation(lnq[:], nq2_ps[:], ACTF.Ln)
            rq = bsb.tile([1, C], F32R)
            nc.scalar.activation(rq[:], lnq[:], ACTF.Exp, scale=-0.5)
            csbc_ps = bsmall.tile([P, C], F32, space=PSUM, tag="bs")
            nc.tensor.matmul(csbc_ps[:], ones_row_r[:], rq[:])
            csbc_sb = bsb.tile([P, C], F32)
            nc.scalar.copy(csbc_sb[:], csbc_ps[:])

            # softmax path: G = Wk^T P1
            g_psl = []
            for m in range(NCHUNK):
                gg = bwork.tile([P, C], F32, space=PSUM, name=f"gps{m}", tag="bw", bufs=4)
                for k in range(NCHUNK):
                    nc.tensor.matmul(
                        gg[:],
                        wqk_r[:, k, ds(C + m * P, P)],
                        p1_sbl[k][:],
                        start=(k == 0),
                        stop=(k == 1),
                    )
                g_psl.append(gg)

            # norm fork #2: Kgram = Wk^T P2, nk2 = diag, rk = nk2^-1/2
            p2_sbl = []
            for m in range(NCHUNK):
                psb = bsb.tile([P, C], F32R, name=f"p2sb{m}", tag="p2sb", bufs=2)
                if m == 0:
                    nc.vector.tensor_copy(psb[:], p2_psl[m][:])
                else:
                    nc.scalar.copy(psb[:], p2_psl[m][:])
                p2_sbl.append(psb)
            nk2 = bsb.tile([P, NCHUNK], F32)
            scrap0 = bsb.tile([P, P], F32)
            scrap1 = bsb.tile([P, P], F32)
            scraps = [scrap0, scrap1]
            for m in range(NCHUNK):
                kg = bwork.tile([P, P], F32, space=PSUM, name=f"kgps{m}", tag="bw", bufs=4)
                for k in range(NCHUNK):
                    nc.tensor.matmul(
                        kg[:],
                        wqk_r[:, k, ds(C + m * P, P)],
                        p2_sbl[k][:, ts(m, P)],
                        start=(k == 0),
                        stop=(k == 1),
                    )
                nc.vector.scalar_tensor_tensor(
                    out=scraps[m][:],
                    in0=kg[:],
                    scalar=1.0,
                    in1=identity_f[:],
                    op0=ALU.mult,
                    op1=ALU.mult,
                    accum_out=nk2[:, m : m + 1],
                )
            lnk = bsb.tile([P, NCHUNK], F32)
            nc.scalar.activation(lnk[:], nk2[:], ACTF.Ln)
            rk = bsb.tile([P, NCHUNK], F32)
            nc.scalar.activation(rk[:], lnk[:], ACTF.Exp, scale=-0.5)

            # A is block-diagonal at chunk level too (heads never span the
            # 128-chunks), so the softmax tail runs on the diagonal 128x128
            # blocks only, and T1[m] = a[m]^T Wp[m] is a single matmul with
            # no cross-chunk dependency.
            t1_sbl = []
            for m in range(NCHUNK):
                dg = ds(m * P, P)
                tt = bsb.tile([P, P], F32, name=f"t{m}", tag="t", bufs=2)
                nc.vector.tensor_mul(tt[:], g_psl[m][:, dg], csbc_sb[:, dg])
                e = bsb.tile([P, P], F32, name=f"e{m}", tag="e", bufs=2)
                nc.scalar.activation(e[:], tt[:], ACTF.Exp, scale=rk[:, m : m + 1])
                em = bsb.tile([P, P], F32, name=f"em{m}", tag="em", bufs=2)
                den = bsb.tile([P, 1], F32, name=f"den{m}", tag="den", bufs=2)
                nc.vector.scalar_tensor_tensor(
                    out=em[:],
                    in0=e[:],
                    scalar=1.0,
                    in1=bdmask[:, m, dg],
                    op0=ALU.mult,
                    op1=ALU.mult,
                    accum_out=den[:],
                )
                rden = bsb.tile([P, 1], F32, name=f"rden{m}", tag="rden", bufs=2)
                nc.vector.reciprocal(rden[:], den[:])
                a_m = bsb.tile([P, P], F32R, name=f"a{m}", tag="a", bufs=2)
                nc.vector.tensor_scalar_mul(a_m[:], em[:], rden[:])
                t1p = bwork.tile([P, C], F32, space=PSUM, name=f"t1ps{m}", tag="bw", bufs=4)
                nc.tensor.matmul(
                    t1p[:], a_m[:], wp_r[:, m, :], start=True, stop=True
                )
                t1s = bsb.tile([P, C], F32R, name=f"t1sb{m}", tag="t1sb", bufs=2)
                if m == 0:
                    nc.vector.tensor_copy(t1s[:], t1p[:])
                else:
                    nc.scalar.copy(t1s[:], t1p[:])
                t1_sbl.append(t1s)

            # Wbig = Wv @ T1  (lhsT = Wv^T chunks)
            for m in range(NCHUNK):
                wbp = bwork.tile([P, C], F32, space=PSUM, name=f"wbps{m}", tag="bw", bufs=4)
                for k in range(NCHUNK):
                    nc.tensor.matmul(
                        wbp[:],
                        wvT[:, k, ts(m, P)],
                        t1_sbl[k][:],
                        start=(k == 0),
                        stop=(k == 1),
                    )
                if m == 0:
                    nc.vector.tensor_copy(wbig_l[m][:], wbp[:])
                else:
                    nc.scalar.copy(wbig_l[m][:], wbp[:])

        s_stack.close()  # free the S PSUM banks for the pass-2 pools

        # ------- pass 2: per group, transpose X tiles then out = X Wbig + bp -------
        bias_v = bias_bc[:].rearrange("p (h c) -> p h c", h=2)
        with tc.tile_pool(name="ops", bufs=4, space=PSUM) as ops, tc.tile_pool(
            name="tpp", bufs=3, space=PSUM
        ) as tpp, tc.tile_pool(name="outb", bufs=4) as outb:

            def emit_transposes(gi):
                # 4 tiles share one PSUM tp tile; a single strided eviction
                # writes all 8 chunk-blocks (one DVE/ACT op per quad)
                gsz = GROUPS[gi]
                for q0 in range(0, gsz, 4):
                    nq = min(4, gsz - q0)
                    tp = tpp.tile([P, 4 * 2 * P], BF16, space=PSUM, tag="tp2")
                    for j in range(nq):
                        t = starts[gi] + q0 + j
                        g, a = divmod(t, DMA_TILES)
                        for k in range(NCHUNK):
                            nc.tensor.transpose(
                                tp[:, ds((j * NCHUNK + k) * P, P)],
                                xg[g][:, a, ts(k, P)],
                                identity[:],
                            )
                    tp_v = tp[:, 0 : nq * NCHUNK * P].rearrange(
                        "p (j k c) -> p k j c", k=NCHUNK, c=P
                    )
                    dst = xTg[gi][:, :, ds(q0 * P, nq * P)].rearrange(
                        "p k (j c) -> p k j c", c=P
                    )
                    if (starts[gi] + q0) % 8 < 4:
                        nc.vector.tensor_copy(dst, tp_v)
                    else:
                        nc.scalar.copy(dst, tp_v)

            pair_idx = 0
            emitted = 0
            for gi, gsz in enumerate(GROUPS):
                t0 = starts[gi]
                # own group's transposes first, deeper lookahead after the
                # out matmuls so the first output DMA isn't delayed
                while emitted <= gi:
                    emit_transposes(emitted)
                    emitted += 1
                ob = outb.tile([P, gsz, C], F32, tag="ob")
                for a2 in range(gsz // 2):
                    o_ps = ops.tile([P, 2 * C], F32, space=PSUM, tag="o")
                    even = pair_idx % 2 == 0
                    for h2 in range(2):
                        j = a2 * 2 + h2
                        for k in range(NCHUNK):
                            nc.tensor.matmul(
                                o_ps[:, ts(h2, C)],
                                xTg[gi][:, k, ts(j, P)],
                                wbig_l[k][:],
                                start=(k == 0),
                                stop=(even and k == 1),
                            )
                        if not even:
                            nc.tensor.matmul(
                                o_ps[:, ts(h2, C)],
                                ones_row_r[:],
                                bp2_r[:, 0:C],
                                start=False,
                                stop=True,
                            )
                    o_v = o_ps[:].rearrange("p (h c) -> p h c", h=2)
                    if even:
                        nc.vector.tensor_add(ob[:, ds(a2 * 2, 2), :], o_v, bias_v)
                    else:
                        nc.scalar.copy(ob[:, ds(a2 * 2, 2), :], o_v)
                    pair_idx += 1
                while emitted <= min(gi + 3, len(GROUPS) - 1):
                    emit_transposes(emitted)
                    emitted += 1
                nc.sync.dma_start(
                    out_dram[ds(t0 * P, gsz * P), :].rearrange(
                        "(a p) c -> p a c", p=P
                    ),
                    ob[:],
                )

    return nc


_NC_CACHE = None


def _get_nc():
    global _NC_CACHE
    if _NC_CACHE is None:
        nc = bacc.Bacc(
            "TRN2",
            target_bir_lowering=False,
            debug=False,
            enable_asserts=False,
            num_devices=N_CORES,
        )
        _build_kernel(nc)
        nc.compile()
        nc.m = get_hw_module(nc.m)
        _NC_CACHE = nc
    return _NC_CACHE


def _make_in_maps(x_in, Wq, Wk, Wv, rescale, Wp, bp):
    x_in = np.ascontiguousarray(np.asarray(x_in, dtype=np.float32))
    maps = []
    for core in range(N_CORES):
        maps.append(
            {
                "x_in": x_in[core].reshape(N, C),
                "Wq": np.asarray(Wq, np.float32),
                "Wk": np.asarray(Wk, np.float32),
                "Wv": np.asarray(Wv, np.float32),
                "rescale": np.asarray(rescale, np.float32),
                "Wp": np.asarray(Wp, np.float32),
                "bp": np.asarray(bp, np.float32),
            }
        )
    return maps


def run_on_hw(inputs: dict, trace: bool = False, tmpdir: str | None = None):
    """Returns (full_output [8,128,128,256] f32, BassKernelResults)."""
    nc = _get_nc()
    in_maps = _make_in_maps(**inputs)
    res = bass_utils.run_bass_kernel_spmd(
        nc, in_maps, core_ids=list(range(N_CORES)), trace=trace, tmpdir=tmpdir
    )
    out = np.stack([res.results[c]["out"].reshape(H, W, C) for c in range(N_CORES)])
    return out.astype(np.float32), res


def kernel(x_in, Wq, Wk, Wv, rescale, Wp, bp) -> np.ndarray:
    out, _ = run_on_hw(
        dict(x_in=x_in, Wq=Wq, Wk=Wk, Wv=Wv, rescale=rescale, Wp=Wp, bp=bp)
    )
    return out



# revision 13
# speedup vs baseline: 1.0440x; 1.0440x over previous
"""Trainium2 Bass kernel for channel-wise ("transposed") attention.

Reference computation (per batch b, X = x_in[b] reshaped [N=16384, C=256]):
    Q = X Wq ; K = X Wk ; V = X Wv            (columns l2-normalized over tokens for Q,K)
    attn[h,i,j] = softmax_j( khat_i . qhat_j * rescale[h] )   (32x32 per head)
    out = (A_bd @ V^T)^T Wp + bp

Algebraic reduction (validated vs reference):
    S    = X^T X                      [256,256]   (only pass-1 reduction needed)
    P1   = S Wq ; P2 = S Wk
    G    = Wk^T P1                    (raw cross-gram K^T Q)
    nq2  = colsum(Wq*rexp^-2 . P1) ; nk2 = diag(Wk^T P2)
    L    = G * rk[i] * (rq*rescale)[j] ;  A = blockdiag-softmax_j(exp(L))
    Wbig = Wv @ (A_bd^T Wp)           [256,256]
    out  = X @ Wbig + bp

Numerics: whole data path in fp16 (not bf16): input X casting-DMA f32->fp16,
weights fp16, S/P1/attention tiles fp16 in SBUF with f32 PSUM accumulation.
fp16's 10-bit mantissa keeps the end-to-end rel err ~5e-4 (vs ~2e-2 with
bf16), and fp16 matmuls/transposes run at the same 1 cyc/row as bf16.

Schedule (per core = one batch, data parallel, no collectives):
  pass 1   stream X f32 -> fp16 SBUF (16 groups of 8 token tiles); PE does the
           symmetric S accumulation (S00|S01 in one 256-wide matmul + S11,
           384 cyc/tile) plus the X-tile transposes (xT, for pass 2) in the
           DMA slack; weight prep matmuls interleaved at fixed points.
  phase B  tiny 256x256 chains -> Wbig, all fp16 matmuls (1 cyc/row).
           Block-diag mask folded into the G PSUM as a rank-5 matmul with
           -B outside blocks, so the softmax row-sum comes free from the
           ACT Exp accumulator.  rsqrt via exp(-0.5 ln x) (act set 6).
  pass 2   out^T = Wbig^T xT + bp computed transposed [C, N]: bias becomes a
           per-partition [P,1] operand fused into the PSUM evictions on both
           DVE and ACT; out streams to DRAM as fp16 (half the DMA bytes),
           host transposes/casts back.
"""

import sys

if "/opt/trn_rl_repo" not in sys.path:
    sys.path.insert(0, "/opt/trn_rl_repo")

from contextlib import ExitStack

import numpy as np

import concourse.bass as bass
import concourse.tile as tile
from concourse import bacc, mybir
from concourse import bass_utils
from concourse.bass import ds, ts
from concourse.bass_interp import get_hw_module
from concourse.masks import make_identity

F32 = mybir.dt.float32
F32R = mybir.dt.float32r
F16 = mybir.dt.float16
ALU = mybir.AluOpType
ACTF = mybir.ActivationFunctionType
PSUM = bass.MemorySpace.PSUM

N_CORES = 8
B, H, W, C = 8, 128, 128, 256
HEADS, DH = 8, 32
N = H * W            # 16384 tokens per batch
P = 128              # partitions / token tile
NT = N // P          # 128 token tiles
DMA_TILES = 8        # token tiles per DMA group
NG = NT // DMA_TILES # 16 groups
NCHUNK = C // P      # 2 channel chunks
QT = 4               # token tiles per transpose/output quad
NQ = NT // QT        # 32 quads
OG = 8               # token tiles per output DMA group
NOG = NT // OG       # 16 output groups

# act_func_sets index of natural_log_exp_and_others: {ln, exp, copy, identity}
ACT_SET_LN_EXP = 6

# Block-diag mask magnitude: logits get -MB outside head blocks before the
# rq/rk normalization scales (~6e-5 combined), leaving ~-32 in the exponent.
MROW = 1024.0
MCOL = 512.0


def _build_kernel(nc: bacc.Bacc):
    x_dram = nc.dram_tensor("x_in", [N, C], F32, kind="ExternalInput").ap()
    wq_dram = nc.dram_tensor("Wq", [C, C], F32, kind="ExternalInput").ap()
    wk_dram = nc.dram_tensor("Wk", [C, C], F32, kind="ExternalInput").ap()
    wv_dram = nc.dram_tensor("Wv", [C, C], F32, kind="ExternalInput").ap()
    resc_dram = nc.dram_tensor("rescale", [HEADS, 1, 1], F32, kind="ExternalInput").ap()
    wp_dram = nc.dram_tensor("Wp", [C, C], F32, kind="ExternalInput").ap()
    bp_dram = nc.dram_tensor("bp", [C], F32, kind="ExternalInput").ap()
    # output is stored transposed [C, N] fp16; host casts + transposes back
    out_dram = nc.dram_tensor("out", [C, N], F16, kind="ExternalOutput").ap()
    outT_v = out_dram.rearrange("(k p) n -> p k n", p=P)

    with tile.TileContext(nc) as tc, ExitStack() as top:
        consts = top.enter_context(tc.tile_pool(name="consts", bufs=1))
        xt_pool = top.enter_context(tc.tile_pool(name="xt", bufs=1))
        xf_pool = top.enter_context(tc.tile_pool(name="xfull", bufs=1))
        s_stack = ExitStack()
        s_pool = s_stack.enter_context(tc.tile_pool(name="spsum", bufs=1, space=PSUM))

        # ------------- const tiles -------------
        identity_f = consts.tile([P, P], F32)
        ident_h = consts.tile([P, P], F16)
        p8 = consts.tile([HEADS, C], F32)
        p8_r = consts.tile([HEADS, C], F32R)
        ones_col = consts.tile([P, 1], F16)
        ones_row = consts.tile([1, P], F32)
        ones_row_r = consts.tile([1, P], F32R)
        ones_row_h = consts.tile([1, P], F16)
        m1024 = consts.tile([1, P], F16)            # blockdiag mask: -B rank-1
        mneg = consts.tile([1, P], F16)
        p8c = consts.tile([P // DH, P], F16)        # +B rank-4 in-block factors
        p8c2 = consts.tile([P // DH, P], F16)

        # weights (fp16 via casting DMA)
        wq_h = consts.tile([P, NCHUNK, C], F16)
        wk_h = consts.tile([P, NCHUNK, C], F16)
        wv_h = consts.tile([P, NCHUNK, C], F16)
        wp_h = consts.tile([P, NCHUNK, C], F16)
        wvT = consts.tile([P, NCHUNK, C], F16)      # wvT[p,q,k] = Wv[k, 128q+p]
        wq_s = consts.tile([P, NCHUNK, C], F16)     # Wq * rexp^-2 (norm fork)
        bp_col = consts.tile([P, NCHUNK], F32)      # bp as per-partition column
        resc_p = consts.tile([HEADS, 1], F32)
        resc_r = consts.tile([HEADS, 1], F32R)
        rexp_row = consts.tile([1, C], F32)         # rescale broadcast over blocks
        rexp1i = consts.tile([1, C], F32)
        rexp2i = consts.tile([1, C], F32)
        wbig = [consts.tile([P, C], F16, name=f"wbig{m}") for m in range(NCHUNK)]

        # X (fp16, resident) and X^T (fp16, built in pass 1)
        xg = [xf_pool.tile([P, DMA_TILES, C], F16, name=f"xg{g}") for g in range(NG)]
        xT = xt_pool.tile([P, NCHUNK, N], F16)

        # S accumulator: [S00|S01] at 0:256, S11 at 256:384 -- one PSUM bank,
        # one zero-region so a single start=True covers both.
        s_ps = s_pool.tile([P, 384], F32, space=PSUM)

        # ---------------- pass 1: load X (fp16), S = X^T X, transposes ----------------
        tp_stack = ExitStack()
        tp_pool = tp_stack.enter_context(tc.tile_pool(name="tp", bufs=2, space=PSUM))
        prep_stack = ExitStack()
        prep_pool = prep_stack.enter_context(
            tc.tile_pool(name="prep", bufs=1, space=PSUM)
        )

        def s_tile(t, first=False, last=False):
            g, a = divmod(t, DMA_TILES)
            x_t = xg[g][:, a, :]
            # symmetric S: [S00|S01] from lhsT=chunk0; S11 from lhsT=chunk1
            nc.tensor.matmul(
                s_ps[:, 0:C], x_t[:, 0:P], x_t[:], start=first, stop=False
            )
            nc.tensor.matmul(
                s_ps[:, C : C + P], x_t[:, P:C], x_t[:, P:C],
                start=False, stop=last,
            )

        emitted_quads = 0

        def emit_quad():
            # transpose 4 token tiles (both chunks) PE->PSUM, evict to xT
            nonlocal emitted_quads
            if emitted_quads >= NQ:
                return
            q = emitted_quads
            emitted_quads += 1
            tp = tp_pool.tile([P, NCHUNK, QT, P], F16, space=PSUM, tag="tp")
            for j in range(QT):
                t = q * QT + j
                g, a = divmod(t, DMA_TILES)
                for k in range(NCHUNK):
                    nc.tensor.transpose(
                        tp[:, k, j, :], xg[g][:, a, ts(k, P)], ident_h[:]
                    )
            for k in range(NCHUNK):
                dst = xT[:, k, ds(q * QT * P, QT * P)].rearrange(
                    "p (j u) -> p j u", u=P
                )
                if (q + k) % 2 == 0:
                    nc.vector.tensor_copy(dst, tp[:, k])
                else:
                    nc.scalar.copy(dst, tp[:, k])

        for g in range(NG):
            if g == 0:
                for lo, n_t in ((0, 4), (4, 4)):
                    nc.gpsimd.dma_start(
                        xg[g][:, ds(lo, n_t), :],
                        x_dram[ds((g * DMA_TILES + lo) * P, n_t * P), :].rearrange(
                            "(a p) c -> p a c", p=P
                        ),
                    )
                # single activation-table load for the whole kernel
                nc.scalar.add_instruction(
                    mybir.InstLoadActFuncSet(
                        name=nc.get_next_instruction_name(),
                        act_func_set_id=ACT_SET_LN_EXP,
                        ins=[],
                        outs=[],
                    )
                )
                make_identity(nc, identity_f[:])
                nc.vector.tensor_copy(ident_h[:], identity_f[:])
                nc.gpsimd.memset(p8[:], 0.0)
                nc.gpsimd.affine_select(
                    out=p8[:].rearrange("p (b i) -> p b i", i=DH),
                    in_=p8[:].rearrange("p (b i) -> p b i", i=DH),
                    compare_op=ALU.not_equal,
                    fill=1.0,
                    base=0,
                    pattern=[[-1, HEADS], [0, DH]],
                    channel_multiplier=1,
                )
                nc.vector.tensor_copy(p8_r[:], p8[:])
                nc.gpsimd.memset(ones_col[:], 1.0)
                nc.gpsimd.memset(ones_row[:], 1.0)
                nc.vector.tensor_copy(ones_row_r[:], ones_row[:])
                nc.vector.tensor_copy(ones_row_h[:], ones_row[:])
                # blockdiag mask factors: -B everywhere (rank 1) + B in-block
                # (rank 4, from the p8 head pattern restricted to one chunk)
                nc.gpsimd.memset(m1024[:], MROW)
                nc.gpsimd.memset(mneg[:], -MCOL)
                nc.vector.tensor_scalar_mul(p8c[:], p8[0 : P // DH, 0:P], MROW)
                nc.vector.tensor_scalar_mul(p8c2[:], p8[0 : P // DH, 0:P], MCOL)
            else:
                nc.gpsimd.dma_start(
                    xg[g][:],
                    x_dram[ds(g * DMA_TILES * P, DMA_TILES * P), :].rearrange(
                        "(a p) c -> p a c", p=P
                    ),
                )
            if g == 1:
                # weight/bias loads (casting f32 -> fp16) behind the first x chunk
                for k in range(NCHUNK):
                    nc.gpsimd.dma_start(wq_h[:, k, :], wq_dram[ts(k, P), :])
                    nc.gpsimd.dma_start(wk_h[:, k, :], wk_dram[ts(k, P), :])
                    nc.gpsimd.dma_start(wv_h[:, k, :], wv_dram[ts(k, P), :])
                    nc.gpsimd.dma_start(wp_h[:, k, :], wp_dram[ts(k, P), :])
                nc.sync.dma_start(
                    bp_col[:], bp_dram.rearrange("(k p) -> p k", p=P)
                )
                nc.sync.dma_start(resc_p[:], resc_dram.rearrange("h a b -> h (a b)"))
                nc.vector.tensor_copy(resc_r[:], resc_p[:])

        # PE stream: S matmuls + prep + transpose quads, ordered so PE never
        # stalls on the input DMA and never goes idle (pstate stays ramped).
        for t in range(0, 4):
            s_tile(t, first=(t == 0))
        # prep block 1: Wv transposes (fp16, one packed PSUM bank), rexp row
        tpv4 = prep_pool.tile([P, 4, P], F16, space=PSUM, tag="tpv")
        for q in range(NCHUNK):
            for m in range(NCHUNK):
                nc.tensor.transpose(
                    tpv4[:, 2 * q + m, :], wv_h[:, m, ts(q, P)], ident_h[:]
                )
        for q in range(NCHUNK):
            dst = wvT[:, q, :].rearrange("p (m u) -> p m u", u=P)
            if q == 0:
                nc.vector.tensor_copy(dst, tpv4[:, ds(2 * q, 2), :])
            else:
                nc.scalar.copy(dst, tpv4[:, ds(2 * q, 2), :])
        rexp_ps = prep_pool.tile([P, C], F32, space=PSUM, tag="bc")
        nc.tensor.matmul(rexp_ps[0:1, :], resc_r[:], p8_r[:], start=True, stop=True)
        nc.scalar.copy(rexp_row[:], rexp_ps[0:1, :])
        nc.vector.reciprocal(rexp1i[:], rexp_row[:])
        nc.vector.tensor_mul(rexp2i[:], rexp1i[:], rexp1i[:])
        for t in range(4, 12):
            s_tile(t)
        # prep block 2: rexp^-2 broadcast + scaled Wq (reuses the bc bank)
        r2bc_ps = prep_pool.tile([P, C], F32, space=PSUM, tag="bc")
        nc.tensor.matmul(r2bc_ps[:], ones_row[:], rexp2i[:], start=True, stop=True)
        for k in range(NCHUNK):
            nc.vector.tensor_mul(wq_s[:, k, :], wq_h[:, k, :], r2bc_ps[:])
        for t in range(12, 16):
            s_tile(t)
        # interleave: after group g's S tiles, transpose quads of group g-1
        for g in range(2, NG):
            for t in range(g * DMA_TILES, (g + 1) * DMA_TILES):
                s_tile(t, last=(t == NT - 1))
            while emitted_quads < (g - 1) * 2:
                emit_quad()
        # quads emitted so far: 28 (tiles 0..111); 4 quads left as phase-B filler

        prep_stack.close()

        # ---------------- phase B: 256x256 attention math (fp16) ----------------
        with tc.tile_pool(name="bwork", bufs=2, space=PSUM) as bwork, tc.tile_pool(
            name="bsmall", bufs=1, space=PSUM
        ) as bsmall, tc.tile_pool(name="bsb", bufs=1) as bsb:
            # S rows chunk0 = [S00|S01]; chunk1 = [S10|S11] with S10 = S01^T
            #   s_row0 = S[0:128, 0:256], s_row1 = S[128:256, 0:256]
            # lhsT for P* chunk (k, m) = S[k-rows, m-cols] = s_row{k}[:, m*128:]
            s_row0 = bsb.tile([P, C], F16)
            s_row1 = bsb.tile([P, C], F16)
            nc.vector.tensor_copy(s_row0[:], s_ps[:, 0:C])
            nc.scalar.copy(s_row1[:, P:C], s_ps[:, C : C + P])
            s10_ps = bsmall.tile([P, P], F16, space=PSUM, tag="bs16")
            nc.tensor.transpose(s10_ps[:], s_row0[:, P:C], ident_h[:])
            nc.vector.tensor_copy(s_row1[:, 0:P], s10_ps[:])

            # 2 filler quads while the S eviction chain completes
            emit_quad()
            emit_quad()

            srows = [s_row0, s_row1]
            # P1 = S Wq, P2 = S Wk -- P1_m and P2_m share one PSUM bank; the
            # zero-region is started by P1's first matmul and stopped by P2's
            # last, so both accumulation groups live in one 2KB bank.
            p12_ps = []
            for m in range(NCHUNK):
                pp = bwork.tile([P, 2 * C], F32, space=PSUM, name=f"p12ps{m}", tag="bw", bufs=2)
                for k in range(NCHUNK):
                    nc.tensor.matmul(
                        pp[:, 0:C], srows[k][:, ts(m, P)], wq_h[:, k, :],
                        start=(k == 0), stop=False,
                    )
                for k in range(NCHUNK):
                    nc.tensor.matmul(
                        pp[:, C : 2 * C], srows[k][:, ts(m, P)], wk_h[:, k, :],
                        start=False, stop=(k == 1),
                    )
                p12_ps.append(pp)

            # evict P1/P2 to fp16; qp for the nq2 fork (reads PSUM directly)
            p1_sb, p2_sb, qpl = [], [], []
            for m in range(NCHUNK):
                psb = bsb.tile([P, C], F16, name=f"p1sb{m}", tag="p1sb", bufs=2)
                if m == 0:
                    nc.vector.tensor_copy(psb[:], p12_ps[m][:, 0:C])
                else:
                    nc.scalar.copy(psb[:], p12_ps[m][:, 0:C])
                p1_sb.append(psb)
                qp = bsb.tile([P, C], F16, name=f"qp{m}", tag="qp", bufs=2)
                nc.vector.tensor_mul(qp[:], wq_s[:, m, :], p12_ps[m][:, 0:C])
                qpl.append(qp)
            for m in range(NCHUNK):
                psb = bsb.tile([P, C], F16, name=f"p2sb{m}", tag="p2sb", bufs=2)
                nc.scalar.copy(psb[:], p12_ps[m][:, C : 2 * C])
                p2_sb.append(psb)

            # 2 filler quads while evictions drain
            emit_quad()
            emit_quad()

            # G (block-diag chunks only) with the rank-5 mask matmul folded in
            # (out-of-block entries get -MROW*MCOL so they vanish in the exp),
            # packed with the Kgram in one bank: [G | Kg | T1] per chunk.
            gkt_ps = []
            for m in range(NCHUNK):
                gg = bwork.tile([P, 2 * C], F32, space=PSUM, name=f"gkt{m}", tag="bw", bufs=2)
                for k in range(NCHUNK):
                    nc.tensor.matmul(
                        gg[:, 0:P], wk_h[:, k, ts(m, P)], p1_sb[k][:, ts(m, P)],
                        start=(k == 0), stop=False,
                    )
                nc.tensor.matmul(gg[:, 0:P], m1024[:], mneg[:], start=False, stop=False)
                nc.tensor.matmul(gg[:, 0:P], p8c[:], p8c2[:], start=False, stop=False)
                gkt_ps.append(gg)

            # nq2 fork: colsum(qp) -> rq' = rsqrt(nq2 * rexp^-2) = rq * rescale
            nq2_ps = bsmall.tile([1, C], F32, space=PSUM, tag="bs")
            for k in range(NCHUNK):
                nc.tensor.matmul(
                    nq2_ps[:], ones_col[:], qpl[k][:], start=(k == 0), stop=(k == 1)
                )
            lnq = bsb.tile([1, C], F32)
            nc.scalar.activation(lnq[:], nq2_ps[:], ACTF.Ln)
            rq_h = bsb.tile([1, C], F16)
            nc.scalar.activation(rq_h[:], lnq[:], ACTF.Exp, scale=-0.5)
            csbc_ps = bsmall.tile([P, C], F32, space=PSUM, tag="bs")
            nc.tensor.matmul(csbc_ps[:], ones_row_h[:], rq_h[:], start=True, stop=True)
            csbc_sb = bsb.tile([P, C], F16)
            nc.vector.tensor_copy(csbc_sb[:], csbc_ps[:])

            # nk2 fork: diag(Wk^T P2) via Kgram + identity-masked row-reduce
            nk2 = bsb.tile([P, NCHUNK], F32)
            scraps = [bsb.tile([P, P], F32, name=f"scrap{m}") for m in range(NCHUNK)]
            for m in range(NCHUNK):
                kg = gkt_ps[m][:, P : 2 * P]
                for k in range(NCHUNK):
                    nc.tensor.matmul(
                        kg, wk_h[:, k, ts(m, P)], p2_sb[k][:, ts(m, P)],
                        start=False, stop=(k == 1),
                    )
                nc.vector.scalar_tensor_tensor(
                    out=scraps[m][:],
                    in0=kg,
                    scalar=1.0,
                    in1=identity_f[:],
                    op0=ALU.mult,
                    op1=ALU.mult,
                    accum_out=nk2[:, m : m + 1],
                )
            lnk = bsb.tile([P, NCHUNK], F32)
            nc.scalar.activation(lnk[:], nk2[:], ACTF.Ln)
            rk = bsb.tile([P, NCHUNK], F32)
            nc.scalar.activation(rk[:], lnk[:], ACTF.Exp, scale=-0.5)

            # 2 filler quads while the norm chains run
            emit_quad()
            emit_quad()

            # softmax tail + T1 + Wbig
            t1_sb = []
            for m in range(NCHUNK):
                tt = bsb.tile([P, P], F16, name=f"t{m}", tag="t", bufs=2)
                nc.vector.tensor_mul(tt[:], gkt_ps[m][:, 0:P], csbc_sb[:, ts(m, P)])
                e = bsb.tile([P, P], F16, name=f"e{m}", tag="e", bufs=2)
                den = bsb.tile([P, 1], F32, name=f"den{m}", tag="den", bufs=2)
                nc.scalar.activation(
                    e[:], tt[:], ACTF.Exp, scale=rk[:, m : m + 1], accum_out=den[:]
                )
                rden = bsb.tile([P, 1], F32, name=f"rden{m}", tag="rden", bufs=2)
                nc.vector.reciprocal(rden[:], den[:])
                a_m = bsb.tile([P, P], F16, name=f"a{m}", tag="a", bufs=2)
                nc.vector.tensor_scalar_mul(a_m[:], e[:], rden[:])
                t1p = bwork.tile(
                    [P, 2 * C], F32, space=PSUM, name=f"t1ps{m}", tag="bw", bufs=2
                )[:, 0:C]
                nc.tensor.matmul(t1p, a_m[:], wp_h[:, m, :], start=True, stop=True)
                t1s = bsb.tile([P, C], F16, name=f"t1sb{m}", tag="t1sb", bufs=2)
                if m == 0:
                    nc.vector.tensor_copy(t1s[:], t1p)
                else:
                    nc.scalar.copy(t1s[:], t1p)
                t1_sb.append(t1s)

            for m in range(NCHUNK):
                wbp = bwork.tile([P, 2 * C], F32, space=PSUM, name=f"wbps{m}", tag="bw", bufs=2)
                wbp = wbp[:, 0:C]
                for q in range(NCHUNK):
                    nc.tensor.matmul(
                        wbp[:], wvT[:, q, ts(m, P)], t1_sb[q][:],
                        start=(q == 0), stop=(q == 1),
                    )
                if m == 0:
                    nc.vector.tensor_copy(wbig[m][:], wbp[:])
                else:
                    nc.scalar.copy(wbig[m][:], wbp[:])

            # remaining transpose quads (if any) while Wbig evicts
            while emitted_quads < NQ:
                emit_quad()

        tp_stack.close()
        s_stack.close()

        # ------- pass 2: out^T = Wbig^T xT + bp, fp16 DMA out -------
        with tc.tile_pool(name="ops", bufs=4, space=PSUM) as ops, tc.tile_pool(
            name="outb", bufs=3
        ) as outb:
            for grp in range(NOG):
                ob = outb.tile([P, NCHUNK, OG * P], F16, tag="ob")
                for half in range(OG // QT):
                    q = grp * (OG // QT) + half
                    tok = ds(q * QT * P, QT * P)
                    for m in range(NCHUNK):
                        o_ps = ops.tile([P, QT * P], F32, space=PSUM, tag="o")
                        for k in range(NCHUNK):
                            nc.tensor.matmul(
                                o_ps[:],
                                wbig[k][:, ts(m, P)],
                                xT[:, k, tok],
                                start=(k == 0),
                                stop=(k == 1),
                            )
                        dst = ob[:, m, ds(half * QT * P, QT * P)]
                        if m == 0:
                            nc.vector.tensor_scalar_add(
                                dst, o_ps[:], bp_col[:, m : m + 1]
                            )
                        else:
                            nc.scalar.activation(
                                dst, o_ps[:], ACTF.Identity,
                                bias=bp_col[:, m : m + 1],
                            )
                nc.sync.dma_start(outT_v[:, :, ds(grp * OG * P, OG * P)], ob[:])

    return nc


_NC_CACHE = None


def _get_nc():
    global _NC_CACHE
    if _NC_CACHE is None:
        nc = bacc.Bacc(
            "TRN2",
            target_bir_lowering=False,
            debug=False,
            enable_asserts=False,
            num_devices=N_CORES,
        )
        _build_kernel(nc)
        nc.compile()
        nc.m = get_hw_module(nc.m)
        _NC_CACHE = nc
    return _NC_CACHE


def _make_in_maps(x_in, Wq, Wk, Wv, rescale, Wp, bp):
    x_in = np.ascontiguousarray(np.asarray(x_in, dtype=np.float32))
    maps = []
    for core in range(N_CORES):
        maps.append(
            {
                "x_in": x_in[core].reshape(N, C),
                "Wq": np.asarray(Wq, np.float32),
                "Wk": np.asarray(Wk, np.float32),
                "Wv": np.asarray(Wv, np.float32),
                "rescale": np.asarray(rescale, np.float32),
                "Wp": np.asarray(Wp, np.float32),
                "bp": np.asarray(bp, np.float32),
            }
        )
    return maps


def run_on_hw(inputs: dict, trace: bool = False, tmpdir: str | None = None):
    """Returns (full_output [8,128,128,256] f32, BassKernelResults)."""
    nc = _get_nc()
    in_maps = _make_in_maps(**inputs)
    res = bass_utils.run_bass_kernel_spmd(
        nc, in_maps, core_ids=list(range(N_CORES)), trace=trace, tmpdir=tmpdir
    )
    out = np.stack(
        [
            np.asarray(res.results[c]["out"], dtype=np.float32).T.reshape(H, W, C)
            for c in range(N_CORES)
        ]
    )
    return out, res


def kernel(x_in, Wq, Wk, Wv, rescale, Wp, bp) -> np.ndarray:
    out, _ = run_on_hw(
        dict(x_in=x_in, Wq=Wq, Wk=Wk, Wv=Wv, rescale=rescale, Wp=Wp, bp=bp)
    )
    return out


# revision 16
# speedup vs baseline: 1.1142x; 1.0672x over previous
"""Trainium2 Bass kernel for channel-wise ("transposed") attention.

Reference computation (per batch b, X = x_in[b] reshaped [N=16384, C=256]):
    Q = X Wq ; K = X Wk ; V = X Wv            (columns l2-normalized over tokens for Q,K)
    attn[h,i,j] = softmax_j( khat_i . qhat_j * rescale[h] )   (32x32 per head)
    out = (A_bd @ V^T)^T Wp + bp

Algebraic reduction (validated vs reference):
    S    = X^T X                      [256,256]   (only pass-1 reduction needed)
    P1   = S Wq ; P2 = S Wk
    G    = Wk^T P1                    (raw cross-gram K^T Q)
    nq2  = colsum(Wq*rexp^-2 . P1) ; nk2 = diag(Wk^T P2)
    L    = G * rk[i] * (rq*rescale)[j] ;  A = blockdiag-softmax_j(exp(L))
    Wbig = Wv @ (A_bd^T Wp)           [256,256]
    out  = X @ Wbig + bp

Numerics: whole data path in fp16 (not bf16): fp16's 10-bit mantissa keeps the
end-to-end rel err ~7e-4 (vs ~2e-2 with bf16) at the same 1 cyc/row matmul
rate.  All accumulation is f32 PSUM.

Schedule (per core = one batch, data parallel, no collectives):
  pass 1   X streams in as fp16 via 8 big casting DMAs on the Pool/SWDGE
           queue (the ~1us SWDGE fixed cost per DMA instruction makes many
           small DMAs Pool-bound).  Tokens are blocked 16-per-partition so
           each DMA needs only 128 descriptors.  Weights load as f32 on the
           SP/HWDGE queue (casting DMAs are Pool-only) and are downcast to
           fp16 by cheap 2x_2p DVE/ACT copies.  PE: symmetric S accumulation
           (S00|S01 fused 256-wide + S11, 384 cyc/tile) with X-tile
           transposes (xT) filling the DMA slack.
  phase B  tiny 256x256 chains -> Wbig, all fp16 matmuls.  The head-block
           mask is folded into the G PSUM as rank-1+rank-4 matmuls with -B
           outside blocks, so the softmax row-sum comes free from the ACT
           Exp accumulator.  rsqrt via exp(-0.5 ln x) (act set 6).
           Leftover transpose quads fill PE stalls.
  pass 2   out^T = Wbig^T xT + bp computed transposed [C, N]: bias is a
           per-partition [P,1] operand fused into the PSUM evictions on both
           DVE and ACT.  Output quads cover contiguous true-token ranges
           (the eviction APs undo the blocked-token permutation), stream out
           as 32 pipelined fp16 DMAs; host transposes/casts back.
"""

import sys

if "/opt/trn_rl_repo" not in sys.path:
    sys.path.insert(0, "/opt/trn_rl_repo")

from contextlib import ExitStack

import numpy as np

import concourse.bass as bass
import concourse.tile as tile
from concourse import bacc, mybir
from concourse import bass_utils
from concourse.bass import ds, ts
from concourse.bass_interp import get_hw_module
from concourse.masks import make_identity

F32 = mybir.dt.float32
F32R = mybir.dt.float32r
F16 = mybir.dt.float16
ALU = mybir.AluOpType
ACTF = mybir.ActivationFunctionType
PSUM = bass.MemorySpace.PSUM

N_CORES = 8
B, H, W, C = 8, 128, 128, 256
HEADS, DH = 8, 32
N = H * W            # 16384 tokens per batch
P = 128              # partitions / token tile
NT = N // P          # 128 token tiles
GT = 16              # token tiles per DMA group (2048 tokens)
NG = NT // GT        # 8 groups
NCHUNK = C // P      # 2 channel chunks
QT = 4               # token tiles per transpose/output quad
NQ = NT // QT        # 32 quads

# act_func_sets index of natural_log_exp_and_others: {ln, exp, copy, identity}
ACT_SET_LN_EXP = 6

# Block-diag mask magnitude: logits get -MROW*MCOL outside head blocks before
# the rq/rk normalization scales (~6e-5 combined), leaving ~-32 in the exp.
MROW = 1024.0
MCOL = 512.0


def _build_kernel(nc: bacc.Bacc):
    x_dram = nc.dram_tensor("x_in", [N, C], F32, kind="ExternalInput").ap()
    wq_dram = nc.dram_tensor("Wq", [C, C], F32, kind="ExternalInput").ap()
    wk_dram = nc.dram_tensor("Wk", [C, C], F32, kind="ExternalInput").ap()
    wv_dram = nc.dram_tensor("Wv", [C, C], F32, kind="ExternalInput").ap()
    resc_dram = nc.dram_tensor("rescale", [HEADS, 1, 1], F32, kind="ExternalInput").ap()
    wp_dram = nc.dram_tensor("Wp", [C, C], F32, kind="ExternalInput").ap()
    bp_dram = nc.dram_tensor("bp", [C], F32, kind="ExternalInput").ap()
    # output is stored transposed [C, N] fp16; host casts + transposes back
    out_dram = nc.dram_tensor("out", [C, N], F16, kind="ExternalOutput").ap()
    outT_v = out_dram.rearrange("(k p) n -> p k n", p=P)

    with tile.TileContext(nc) as tc, ExitStack() as top:
        consts = top.enter_context(tc.tile_pool(name="consts", bufs=1))
        xt_pool = top.enter_context(tc.tile_pool(name="xt", bufs=1))
        xf_pool = top.enter_context(tc.tile_pool(name="xfull", bufs=1))
        # PSUM pool stack (LIFO dealloc): tp (lives through pass 2) ->
        # spsum (closed early in phase B) -> prep (closed end of pass 1)
        tp_stack = ExitStack()
        tp_pool = tp_stack.enter_context(tc.tile_pool(name="tp", bufs=2, space=PSUM))
        s_stack = ExitStack()
        s_pool = s_stack.enter_context(tc.tile_pool(name="spsum", bufs=1, space=PSUM))
        prep_stack = ExitStack()
        prep_pool = prep_stack.enter_context(
            tc.tile_pool(name="prep", bufs=1, space=PSUM)
        )

        # ------------- const tiles -------------
        identity_f = consts.tile([P, P], F32)
        ident_h = consts.tile([P, P], F16)
        p8 = consts.tile([HEADS, C], F32)
        p8_r = consts.tile([HEADS, C], F32R)
        ones_col = consts.tile([P, 1], F16)
        ones_row = consts.tile([1, P], F32)
        ones_row_h = consts.tile([1, P], F16)
        m1024 = consts.tile([1, P], F16)            # blockdiag mask: -B rank-1
        mneg = consts.tile([1, P], F16)
        p8c = consts.tile([P // DH, P], F16)        # +B rank-4 in-block factors
        p8c2 = consts.tile([P // DH, P], F16)

        # weights: f32 staging (HWDGE DMA), fp16 working copies
        wq_f = consts.tile([P, NCHUNK, C], F32)
        wk_f = consts.tile([P, NCHUNK, C], F32)
        wv_f = consts.tile([P, NCHUNK, C], F32)
        wp_f = consts.tile([P, NCHUNK, C], F32)
        wq_h = consts.tile([P, NCHUNK, C], F16)
        wk_h = consts.tile([P, NCHUNK, C], F16)
        wv_h = consts.tile([P, NCHUNK, C], F16)
        wp_h = consts.tile([P, NCHUNK, C], F16)
        wvT = consts.tile([P, NCHUNK, C], F16)      # wvT[p,q,k] = Wv[k, 128q+p]
        wq_s = consts.tile([P, NCHUNK, C], F16)     # Wq * rexp^-2 (norm fork)
        bp_col = consts.tile([P, NCHUNK], F32)      # bp as per-partition column
        resc_p = consts.tile([HEADS, 1], F32)
        resc_r = consts.tile([HEADS, 1], F32R)
        rexp_row = consts.tile([1, C], F32)         # rescale broadcast over blocks
        rexp1i = consts.tile([1, C], F32)
        rexp2i = consts.tile([1, C], F32)
        wbig = [consts.tile([P, C], F16, name=f"wbig{m}") for m in range(NCHUNK)]

        # X (fp16, resident, blocked 16 tokens/partition) and X^T (fp16).
        # xg[g][p, j, :] = x[g*2048 + 16*p + j, :]   (tile (g,j) = tokens
        # {16p+j}); xT[:, k, 128*t + u] = tile t's transpose column u, i.e.
        # token g*2048 + 16*u + j for t = g*16 + j.
        xg = [xf_pool.tile([P, GT, C], F16, name=f"xg{g}") for g in range(NG)]
        xT = xt_pool.tile([P, NCHUNK, N], F16)

        # S accumulator: [S00|S01] at 0:256, S11 at 256:384 -- one PSUM bank,
        # one zero-region so a single start=True covers both.
        s_ps = s_pool.tile([P, 384], F32, space=PSUM)

        # ---------------- pass 1: load X (fp16), S = X^T X, transposes ----------------
        def s_tile(t, first=False, last=False):
            g, a = divmod(t, GT)
            x_t = xg[g][:, a, :]
            # symmetric S: [S00|S01] from lhsT=chunk0; S11 from lhsT=chunk1
            nc.tensor.matmul(
                s_ps[:, 0:C], x_t[:, 0:P], x_t[:], start=first, stop=False
            )
            nc.tensor.matmul(
                s_ps[:, C : C + P], x_t[:, P:C], x_t[:, P:C],
                start=False, stop=last,
            )

        emitted_quads = 0

        def emit_quad():
            # transpose 4 token tiles (both chunks) PE->PSUM, evict to xT
            nonlocal emitted_quads
            if emitted_quads >= NQ:
                return
            q = emitted_quads
            emitted_quads += 1
            tp = tp_pool.tile([P, NCHUNK, QT, P], F16, space=PSUM, tag="tp")
            for j in range(QT):
                t = q * QT + j
                g, a = divmod(t, GT)
                for k in range(NCHUNK):
                    nc.tensor.transpose(
                        tp[:, k, j, :], xg[g][:, a, ts(k, P)], ident_h[:]
                    )
            for k in range(NCHUNK):
                dst = xT[:, k, ds(q * QT * P, QT * P)].rearrange(
                    "p (j u) -> p j u", u=P
                )
                if (q + k) % 2 == 0:
                    nc.vector.tensor_copy(dst, tp[:, k])
                else:
                    nc.scalar.copy(dst, tp[:, k])

        def x_dma(g, j0, j1):
            nc.gpsimd.dma_start(
                xg[g][:, ds(j0, j1 - j0), :],
                x_dram[ds(g * GT * P, GT * P), :].rearrange(
                    "(p j) c -> p j c", j=GT
                )[:, ds(j0, j1 - j0), :],
            )

        for g in range(NG):
            if g == 0:
                for j0, j1 in ((0, 2), (2, 8), (8, GT)):
                    x_dma(g, j0, j1)
                # single activation-table load for the whole kernel
                nc.scalar.add_instruction(
                    mybir.InstLoadActFuncSet(
                        name=nc.get_next_instruction_name(),
                        act_func_set_id=ACT_SET_LN_EXP,
                        ins=[],
                        outs=[],
                    )
                )
                make_identity(nc, identity_f[:])
                nc.vector.tensor_copy(ident_h[:], identity_f[:])
                nc.gpsimd.memset(p8[:], 0.0)
                nc.gpsimd.affine_select(
                    out=p8[:].rearrange("p (b i) -> p b i", i=DH),
                    in_=p8[:].rearrange("p (b i) -> p b i", i=DH),
                    compare_op=ALU.not_equal,
                    fill=1.0,
                    base=0,
                    pattern=[[-1, HEADS], [0, DH]],
                    channel_multiplier=1,
                )
                nc.vector.tensor_copy(p8_r[:], p8[:])
                nc.gpsimd.memset(ones_col[:], 1.0)
                nc.gpsimd.memset(ones_row[:], 1.0)
                nc.vector.tensor_copy(ones_row_h[:], ones_row[:])
                # blockdiag mask factors: -B everywhere (rank 1) + B in-block
                # (rank 4, from the p8 head pattern restricted to one chunk)
                nc.gpsimd.memset(m1024[:], MROW)
                nc.gpsimd.memset(mneg[:], -MCOL)
                nc.vector.tensor_scalar_mul(p8c[:], p8[0 : P // DH, 0:P], MROW)
                nc.vector.tensor_scalar_mul(p8c2[:], p8[0 : P // DH, 0:P], MCOL)
            else:
                x_dma(g, 0, GT)
            if g == 1:
                # weights as f32 on the SP/HWDGE queue (cast DMAs are
                # Pool-only and each SWDGE DMA costs ~1us of Pool time)
                for wf, wd in (
                    (wq_f, wq_dram), (wk_f, wk_dram),
                    (wv_f, wv_dram), (wp_f, wp_dram),
                ):
                    for k in range(NCHUNK):
                        nc.sync.dma_start(wf[:, k, :], wd[ts(k, P), :])
                nc.sync.dma_start(
                    bp_col[:], bp_dram.rearrange("(k p) -> p k", p=P)
                )
                nc.sync.dma_start(resc_p[:], resc_dram.rearrange("h a b -> h (a b)"))
                nc.vector.tensor_copy(resc_r[:], resc_p[:])
                # fp16 working copies (2x_2p SBUF->SBUF copies, ~330ns each)
                nc.vector.tensor_copy(wv_h[:], wv_f[:])
                nc.scalar.copy(wq_h[:], wq_f[:])
                nc.vector.tensor_copy(wk_h[:], wk_f[:])
                nc.scalar.copy(wp_h[:], wp_f[:])

        # PE stream: S matmuls + prep + transpose quads.  Pass-1 PE is nearly
        # DMA-paced; one quad per group fills the slack.
        for t in range(0, 4):
            s_tile(t, first=(t == 0))
        # prep block 1: Wv transposes (fp16, one packed PSUM bank), rexp row
        tpv4 = prep_pool.tile([P, 4, P], F16, space=PSUM, tag="tpv")
        for q in range(NCHUNK):
            for m in range(NCHUNK):
                nc.tensor.transpose(
                    tpv4[:, 2 * q + m, :], wv_h[:, m, ts(q, P)], ident_h[:]
                )
        for q in range(NCHUNK):
            dst = wvT[:, q, :].rearrange("p (m u) -> p m u", u=P)
            if q == 0:
                nc.vector.tensor_copy(dst, tpv4[:, ds(2 * q, 2), :])
            else:
                nc.scalar.copy(dst, tpv4[:, ds(2 * q, 2), :])
        rexp_ps = prep_pool.tile([P, C], F32, space=PSUM, tag="bc")
        nc.tensor.matmul(rexp_ps[0:1, :], resc_r[:], p8_r[:], start=True, stop=True)
        nc.scalar.copy(rexp_row[:], rexp_ps[0:1, :])
        nc.vector.reciprocal(rexp1i[:], rexp_row[:])
        nc.vector.tensor_mul(rexp2i[:], rexp1i[:], rexp1i[:])
        for t in range(4, 12):
            s_tile(t)
        # prep block 2: rexp^-2 broadcast + scaled Wq (reuses the bc bank)
        r2bc_ps = prep_pool.tile([P, C], F32, space=PSUM, tag="bc")
        nc.tensor.matmul(r2bc_ps[:], ones_row[:], rexp2i[:], start=True, stop=True)
        for k in range(NCHUNK):
            nc.vector.tensor_mul(wq_s[:, k, :], wq_h[:, k, :], r2bc_ps[:])
        for t in range(12, 16):
            s_tile(t)
        prep_stack.close()  # tpv/bc banks free from here
        emit_quad()
        for g in range(1, NG):
            for t in range(g * GT, (g + 1) * GT):
                s_tile(t, last=(t == NT - 1))
            emit_quad()

        # ---------------- phase B: 256x256 attention math (fp16) ----------------
        # S rows chunk0 = [S00|S01]; chunk1 = [S10|S11] with S10 = S01^T
        #   s_row0 = S[0:128, 0:256], s_row1 = S[128:256, 0:256]
        # lhsT for P* chunk (k, m) = S[k-rows, m-cols] = s_row{k}[:, m*128:]
        with tc.tile_pool(name="bsb0", bufs=1) as bsb0:
            s_row0 = bsb0.tile([P, C], F16)
            s_row1 = bsb0.tile([P, C], F16)
            nc.vector.tensor_copy(s_row0[:], s_ps[:, 0:C])
            nc.scalar.copy(s_row1[:, P:C], s_ps[:, C : C + P])
            with tc.tile_pool(name="preb", bufs=1, space=PSUM) as pre_b:
                s10_ps = pre_b.tile([P, P], F16, space=PSUM, tag="bs16")
                nc.tensor.transpose(s10_ps[:], s_row0[:, P:C], ident_h[:])
                nc.vector.tensor_copy(s_row1[:, 0:P], s10_ps[:])
            s_stack.close()  # S bank free from here on

            emit_quad()
            emit_quad()

            srows = [s_row0, s_row1]
            bwork_ctx = ExitStack()
            bwork = bwork_ctx.enter_context(
                tc.tile_pool(name="bwork", bufs=4, space=PSUM)
            )
            bsmall = bwork_ctx.enter_context(
                tc.tile_pool(name="bsmall", bufs=1, space=PSUM)
            )
            bsb = bwork_ctx.enter_context(tc.tile_pool(name="bsb", bufs=1))
            # P1 = S Wq, P2 = S Wk
            p1_ps, p2_ps = [], []
            for dst_list, w_h in ((p1_ps, wq_h), (p2_ps, wk_h)):
                for m in range(NCHUNK):
                    pp = bwork.tile(
                        [P, C], F32, space=PSUM,
                        name=f"pps{len(dst_list)}{m}", tag="bw", bufs=4,
                    )
                    for k in range(NCHUNK):
                        nc.tensor.matmul(
                            pp[:], srows[k][:, ts(m, P)], w_h[:, k, :],
                            start=(k == 0), stop=(k == 1),
                        )
                    dst_list.append(pp)

            # evict P1/P2 to fp16; qp for the nq2 fork (reads PSUM directly)
            p1_sb, p2_sb, qpl = [], [], []
            for m in range(NCHUNK):
                psb = bsb.tile([P, C], F16, name=f"p1sb{m}", tag="p1sb", bufs=2)
                if m == 0:
                    nc.vector.tensor_copy(psb[:], p1_ps[m][:])
                else:
                    nc.scalar.copy(psb[:], p1_ps[m][:])
                p1_sb.append(psb)
                qp = bsb.tile([P, C], F16, name=f"qp{m}", tag="qp", bufs=2)
                nc.vector.tensor_mul(qp[:], wq_s[:, m, :], p1_ps[m][:])
                qpl.append(qp)
            for m in range(NCHUNK):
                psb = bsb.tile([P, C], F16, name=f"p2sb{m}", tag="p2sb", bufs=2)
                nc.scalar.copy(psb[:], p2_ps[m][:])
                p2_sb.append(psb)

            emit_quad()
            emit_quad()

            # G (block-diag chunks only) with the mask matmuls folded in:
            # out-of-block entries get -MROW*MCOL so they vanish in the exp.
            g_ps = []
            for m in range(NCHUNK):
                gg = bwork.tile([P, P], F32, space=PSUM, name=f"gps{m}", tag="bw", bufs=4)
                for k in range(NCHUNK):
                    nc.tensor.matmul(
                        gg[:], wk_h[:, k, ts(m, P)], p1_sb[k][:, ts(m, P)],
                        start=(k == 0), stop=False,
                    )
                nc.tensor.matmul(gg[:], m1024[:], mneg[:], start=False, stop=False)
                nc.tensor.matmul(gg[:], p8c[:], p8c2[:], start=False, stop=True)
                g_ps.append(gg)

            # nq2 fork: colsum(qp) -> rq' = rsqrt(nq2 * rexp^-2) = rq * rescale
            nq2_ps = bsmall.tile([1, C], F32, space=PSUM, tag="bs")
            for k in range(NCHUNK):
                nc.tensor.matmul(
                    nq2_ps[:], ones_col[:], qpl[k][:], start=(k == 0), stop=(k == 1)
                )
            lnq = bsb.tile([1, C], F32)
            nc.scalar.activation(lnq[:], nq2_ps[:], ACTF.Ln)
            rq_h = bsb.tile([1, C], F16)
            nc.scalar.activation(rq_h[:], lnq[:], ACTF.Exp, scale=-0.5)
            csbc_ps = bsmall.tile([P, C], F32, space=PSUM, tag="bs")
            nc.tensor.matmul(csbc_ps[:], ones_row_h[:], rq_h[:], start=True, stop=True)
            csbc_sb = bsb.tile([P, C], F16)
            nc.vector.tensor_copy(csbc_sb[:], csbc_ps[:])

            # nk2 fork: diag(Wk^T P2) via Kgram + identity-masked row-reduce
            nk2 = bsb.tile([P, NCHUNK], F32)
            scraps = [bsb.tile([P, P], F32, name=f"scrap{m}") for m in range(NCHUNK)]
            for m in range(NCHUNK):
                kg = bwork.tile([P, P], F32, space=PSUM, name=f"kg{m}", tag="bw", bufs=4)
                for k in range(NCHUNK):
                    nc.tensor.matmul(
                        kg[:], wk_h[:, k, ts(m, P)], p2_sb[k][:, ts(m, P)],
                        start=(k == 0), stop=(k == 1),
                    )
                nc.vector.scalar_tensor_tensor(
                    out=scraps[m][:],
                    in0=kg[:],
                    scalar=1.0,
                    in1=identity_f[:],
                    op0=ALU.mult,
                    op1=ALU.mult,
                    accum_out=nk2[:, m : m + 1],
                )
            lnk = bsb.tile([P, NCHUNK], F32)
            nc.scalar.activation(lnk[:], nk2[:], ACTF.Ln)
            rk = bsb.tile([P, NCHUNK], F32)
            nc.scalar.activation(rk[:], lnk[:], ACTF.Exp, scale=-0.5)

            emit_quad()
            emit_quad()

            # softmax tail + T1 + Wbig
            t1_sb = []
            for m in range(NCHUNK):
                tt = bsb.tile([P, P], F16, name=f"t{m}", tag="t", bufs=2)
                nc.vector.tensor_mul(tt[:], g_ps[m][:], csbc_sb[:, ts(m, P)])
                e = bsb.tile([P, P], F16, name=f"e{m}", tag="e", bufs=2)
                den = bsb.tile([P, 1], F32, name=f"den{m}", tag="den", bufs=2)
                nc.scalar.activation(
                    e[:], tt[:], ACTF.Exp, scale=rk[:, m : m + 1], accum_out=den[:]
                )
                rden = bsb.tile([P, 1], F32, name=f"rden{m}", tag="rden", bufs=2)
                nc.vector.reciprocal(rden[:], den[:])
                a_m = bsb.tile([P, P], F16, name=f"a{m}", tag="a", bufs=2)
                nc.vector.tensor_scalar_mul(a_m[:], e[:], rden[:])
                t1p = bwork.tile(
                    [P, C], F32, space=PSUM, name=f"t1ps{m}", tag="bw", bufs=4
                )
                nc.tensor.matmul(t1p[:], a_m[:], wp_h[:, m, :], start=True, stop=True)
                t1s = bsb.tile([P, C], F16, name=f"t1sb{m}", tag="t1sb", bufs=2)
                if m == 0:
                    nc.vector.tensor_copy(t1s[:], t1p[:])
                else:
                    nc.scalar.copy(t1s[:], t1p[:])
                t1_sb.append(t1s)

            for m in range(NCHUNK):
                wbp = bwork.tile(
                    [P, C], F32, space=PSUM, name=f"wbps{m}", tag="bw", bufs=4
                )
                for q in range(NCHUNK):
                    nc.tensor.matmul(
                        wbp[:], wvT[:, q, ts(m, P)], t1_sb[q][:],
                        start=(q == 0), stop=(q == 1),
                    )
                if m == 0:
                    nc.vector.tensor_copy(wbig[m][:], wbp[:])
                else:
                    nc.scalar.copy(wbig[m][:], wbp[:])
            bwork_ctx.close()

        # ------- pass 2: out^T = Wbig^T xT + bp, 32 pipelined fp16 DMAs -------
        # Output quad oq covers TRUE tokens [oq*512, (oq+1)*512): group
        # g = oq//4, u in [32*(oq%4), +32), all j in [0,16).  The matmul rhs
        # gathers the scattered xT positions; the eviction AP un-permutes
        # (j,u) -> 16u+j so each DMA writes a contiguous token range.
        with tc.tile_pool(name="ops", bufs=4, space=PSUM) as ops, tc.tile_pool(
            name="outb", bufs=4
        ) as outb:
            for oq in range(NQ):
                g, uq = divmod(oq, NQ // NG)
                # all transposes of group g must be available
                while emitted_quads < (g + 1) * (NQ // NG):
                    emit_quad()
                ob = outb.tile([P, NCHUNK, QT * P], F16, tag="ob")
                for m in range(NCHUNK):
                    o_ps = ops.tile([P, QT * P], F32, space=PSUM, tag="o")
                    for k in range(NCHUNK):
                        rhs = xT[:, k, ds(g * GT * P, GT * P)].rearrange(
                            "p (j u) -> p j u", u=P
                        )[:, :, ds(uq * 32, 32)]
                        nc.tensor.matmul(
                            o_ps[:].rearrange("p (j u) -> p j u", u=32),
                            wbig[k][:, ts(m, P)],
                            rhs,
                            start=(k == 0),
                            stop=(k == 1),
                        )
                    # evict + bias; o_ps columns are (j, u), true token
                    # offset within the quad is 16u + j
                    dst = ob[:, m, :].rearrange("p (u j) -> p u j", j=GT)
                    src = o_ps[:].rearrange("p (j u) -> p u j", u=32)
                    if m == 0:
                        nc.vector.tensor_scalar_add(dst, src, bp_col[:, m : m + 1])
                    else:
                        nc.scalar.activation(
                            dst, src, ACTF.Identity, bias=bp_col[:, m : m + 1]
                        )
                nc.sync.dma_start(outT_v[:, :, ds(oq * QT * P, QT * P)], ob[:])

        tp_stack.close()

    return nc


_NC_CACHE = None


def _get_nc():
    global _NC_CACHE
    if _NC_CACHE is None:
        nc = bacc.Bacc(
            "TRN2",
            target_bir_lowering=False,
            debug=False,
            enable_asserts=False,
            num_devices=N_CORES,
        )
        _build_kernel(nc)
        nc.compile()
        nc.m = get_hw_module(nc.m)
        _NC_CACHE = nc
    return _NC_CACHE


def _make_in_maps(x_in, Wq, Wk, Wv, rescale, Wp, bp):
    x_in = np.ascontiguousarray(np.asarray(x_in, dtype=np.float32))
    maps = []
    for core in range(N_CORES):
        maps.append(
            {
                "x_in": x_in[core].reshape(N, C),
                "Wq": np.asarray(Wq, np.float32),
                "Wk": np.asarray(Wk, np.float32),
                "Wv": np.asarray(Wv, np.float32),
                "rescale": np.asarray(rescale, np.float32),
                "Wp": np.asarray(Wp, np.float32),
                "bp": np.asarray(bp, np.float32),
            }
        )
    return maps


def run_on_hw(inputs: dict, trace: bool = False, tmpdir: str | None = None):
    """Returns (full_output [8,128,128,256] f32, BassKernelResults)."""
    nc = _get_nc()
    in_maps = _make_in_maps(**inputs)
    res = bass_utils.run_bass_kernel_spmd(
        nc, in_maps, core_ids=list(range(N_CORES)), trace=trace, tmpdir=tmpdir
    )
    out = np.stack(
        [
            np.asarray(res.results[c]["out"], dtype=np.float32).T.reshape(H, W, C)
            for c in range(N_CORES)
        ]
    )
    return out, res


def kernel(x_in, Wq, Wk, Wv, rescale, Wp, bp) -> np.ndarray:
    out, _ = run_on_hw(
        dict(x_in=x_in, Wq=Wq, Wk=Wk, Wv=Wv, rescale=rescale, Wp=Wp, bp=bp)
    )
    return out


# revision 17
# speedup vs baseline: 1.1569x; 1.0383x over previous
"""Trainium2 Bass kernel for channel-wise ("transposed") attention.

Reference computation (per batch b, X = x_in[b] reshaped [N=16384, C=256]):
    Q = X Wq ; K = X Wk ; V = X Wv            (columns l2-normalized over tokens for Q,K)
    attn[h,i,j] = softmax_j( khat_i . qhat_j * rescale[h] )   (32x32 per head)
    out = (A_bd @ V^T)^T Wp + bp

Algebraic reduction (validated vs reference):
    S    = X^T X                      [256,256]   (only pass-1 reduction needed)
    P1   = S Wq ; P2 = S Wk
    G    = Wk^T P1                    (raw cross-gram K^T Q)
    nq2  = colsum(Wq*rexp^-2 . P1) ; nk2 = diag(Wk^T P2)
    L    = G * rk[i] * (rq*rescale)[j] ;  A = blockdiag-softmax_j(exp(L))
    Wbig = Wv @ (A_bd^T Wp)           [256,256]
    out  = X @ Wbig + bp

Numerics: whole data path in fp16 (not bf16): fp16's 10-bit mantissa keeps the
end-to-end rel err ~7e-4 (vs ~2e-2 with bf16) at the same 1 cyc/row matmul
rate.  All accumulation is f32 PSUM.

Schedule (per core = one batch, data parallel, no collectives):
  pass 1   X streams in as fp16 via 8 big casting DMAs on the Pool/SWDGE
           queue (the ~1us SWDGE fixed cost per DMA instruction makes many
           small DMAs Pool-bound).  Tokens are blocked 16-per-partition so
           each DMA needs only 128 descriptors.  Weights load as f32 on the
           SP/HWDGE queue (casting DMAs are Pool-only) and are downcast to
           fp16 by cheap 2x_2p DVE/ACT copies.  PE: symmetric S accumulation
           (S00|S01 fused 256-wide + S11, 384 cyc/tile) with X-tile
           transposes (xT) filling the DMA slack.
  phase B  tiny 256x256 chains -> Wbig, all fp16 matmuls.  The head-block
           mask is folded into the G PSUM as rank-1+rank-4 matmuls with -B
           outside blocks, so the softmax row-sum comes free from the ACT
           Exp accumulator.  rsqrt via exp(-0.5 ln x) (act set 6).
           Leftover transpose quads fill PE stalls.
  pass 2   out^T = Wbig^T xT + bp computed transposed [C, N]: bias is a
           per-partition [P,1] operand fused into the PSUM evictions on both
           DVE and ACT.  Output quads cover contiguous true-token ranges
           (the eviction APs undo the blocked-token permutation), stream out
           as 32 pipelined fp16 DMAs; host transposes/casts back.
"""

import sys

if "/opt/trn_rl_repo" not in sys.path:
    sys.path.insert(0, "/opt/trn_rl_repo")

from contextlib import ExitStack

import numpy as np

import concourse.bass as bass
import concourse.tile as tile
from concourse import bacc, mybir
from concourse import bass_utils
from concourse.bass import ds, ts
from concourse.bass_interp import get_hw_module
from concourse.masks import make_identity

F32 = mybir.dt.float32
F32R = mybir.dt.float32r
F16 = mybir.dt.float16
ALU = mybir.AluOpType
ACTF = mybir.ActivationFunctionType
PSUM = bass.MemorySpace.PSUM

N_CORES = 8
B, H, W, C = 8, 128, 128, 256
HEADS, DH = 8, 32
N = H * W            # 16384 tokens per batch
P = 128              # partitions / token tile
NT = N // P          # 128 token tiles
GT = 16              # token tiles per DMA group (2048 tokens)
NG = NT // GT        # 8 groups
NCHUNK = C // P      # 2 channel chunks
QT = 4               # token tiles per transpose/output quad
NQ = NT // QT        # 32 quads

# act_func_sets index of natural_log_exp_and_others: {ln, exp, copy, identity}
ACT_SET_LN_EXP = 6

# Block-diag mask magnitude: logits get -MROW*MCOL outside head blocks before
# the rq/rk normalization scales (~6e-5 combined), leaving ~-32 in the exp.
MROW = 1024.0
MCOL = 512.0


def _build_kernel(nc: bacc.Bacc):
    x_dram = nc.dram_tensor("x_in", [N, C], F32, kind="ExternalInput").ap()
    wq_dram = nc.dram_tensor("Wq", [C, C], F32, kind="ExternalInput").ap()
    wk_dram = nc.dram_tensor("Wk", [C, C], F32, kind="ExternalInput").ap()
    wv_dram = nc.dram_tensor("Wv", [C, C], F32, kind="ExternalInput").ap()
    resc_dram = nc.dram_tensor("rescale", [HEADS, 1, 1], F32, kind="ExternalInput").ap()
    wp_dram = nc.dram_tensor("Wp", [C, C], F32, kind="ExternalInput").ap()
    bp_dram = nc.dram_tensor("bp", [C], F32, kind="ExternalInput").ap()
    # output is stored transposed [C, N] fp16; host casts + transposes back
    out_dram = nc.dram_tensor("out", [C, N], F16, kind="ExternalOutput").ap()
    outT_v = out_dram.rearrange("(k p) n -> p k n", p=P)

    with tile.TileContext(nc) as tc, ExitStack() as top:
        consts = top.enter_context(tc.tile_pool(name="consts", bufs=1))
        xt_pool = top.enter_context(tc.tile_pool(name="xt", bufs=1))
        xf_pool = top.enter_context(tc.tile_pool(name="xfull", bufs=1))
        # PSUM pool stack (LIFO dealloc): tp (lives through pass 2) ->
        # spsum (closed early in phase B) -> prep (closed end of pass 1)
        tp_stack = ExitStack()
        tp_pool = tp_stack.enter_context(tc.tile_pool(name="tp", bufs=2, space=PSUM))
        s_stack = ExitStack()
        s_pool = s_stack.enter_context(tc.tile_pool(name="spsum", bufs=1, space=PSUM))
        prep_stack = ExitStack()
        prep_pool = prep_stack.enter_context(
            tc.tile_pool(name="prep", bufs=1, space=PSUM)
        )

        # ------------- const tiles -------------
        identity_f = consts.tile([P, P], F32)
        ident_h = consts.tile([P, P], F16)
        p8 = consts.tile([HEADS, C], F32)
        p8_r = consts.tile([HEADS, C], F32R)
        ones_col = consts.tile([P, 1], F16)
        ones_row = consts.tile([1, P], F32)
        ones_row_h = consts.tile([1, P], F16)
        m1024 = consts.tile([1, P], F16)            # blockdiag mask: -B rank-1
        mneg = consts.tile([1, P], F16)
        p8c = consts.tile([P // DH, P], F16)        # +B rank-4 in-block factors
        p8c2 = consts.tile([P // DH, P], F16)

        # weights (fp16 via gpsimd casting DMA, one per weight)
        wq_h = consts.tile([P, NCHUNK, C], F16)
        wk_h = consts.tile([P, NCHUNK, C], F16)
        wv_h = consts.tile([P, NCHUNK, C], F16)
        wp_h = consts.tile([P, NCHUNK, C], F16)
        wvT = consts.tile([P, NCHUNK, C], F16)      # wvT[p,q,k] = Wv[k, 128q+p]
        wq_s = consts.tile([P, NCHUNK, C], F16)     # Wq * rexp^-2 (norm fork)
        bp_col = consts.tile([P, NCHUNK], F32)      # bp as per-partition column
        resc_p = consts.tile([HEADS, 1], F32)
        resc_r = consts.tile([HEADS, 1], F32R)
        rexp_row = consts.tile([1, C], F32)         # rescale broadcast over blocks
        rexp1i = consts.tile([1, C], F32)
        rexp2i = consts.tile([1, C], F32)
        wbig = [consts.tile([P, C], F16, name=f"wbig{m}") for m in range(NCHUNK)]

        # X (fp16, resident, blocked 16 tokens/partition) and X^T (fp16).
        # xg[g][p, j, :] = x[g*2048 + 16*p + j, :]   (tile (g,j) = tokens
        # {16p+j}); xT[:, k, 128*t + u] = tile t's transpose column u, i.e.
        # token g*2048 + 16*u + j for t = g*16 + j.
        xg = [xf_pool.tile([P, GT, C], F16, name=f"xg{g}") for g in range(NG)]
        xT = xt_pool.tile([P, NCHUNK, N], F16)

        # S accumulator: [S00|S01] at 0:256, S11 at 256:384 -- one PSUM bank,
        # one zero-region so a single start=True covers both.
        s_ps = s_pool.tile([P, 384], F32, space=PSUM)

        # ---------------- pass 1: load X (fp16), S = X^T X, transposes ----------------
        def s_tile(t, first=False, last=False):
            g, a = divmod(t, GT)
            x_t = xg[g][:, a, :]
            # symmetric S: [S00|S01] from lhsT=chunk0; S11 from lhsT=chunk1
            nc.tensor.matmul(
                s_ps[:, 0:C], x_t[:, 0:P], x_t[:], start=first, stop=False
            )
            nc.tensor.matmul(
                s_ps[:, C : C + P], x_t[:, P:C], x_t[:, P:C],
                start=False, stop=last,
            )

        emitted_quads = 0

        def dummies(n):
            # dependency-free PE filler: keeps the pstate ramp alive across
            # known stall windows (any PE idle resets the 3us ramp clock)
            for _ in range(n):
                dscr = tp_pool.tile([P, P], F32, space=PSUM, tag="dum", bufs=1)
                nc.tensor.matmul(
                    dscr[:], ones_row_h[:], ones_row_h[:], start=True, stop=True
                )

        def emit_quad():
            # transpose 4 token tiles (both chunks) PE->PSUM, evict to xT
            nonlocal emitted_quads
            if emitted_quads >= NQ:
                return
            q = emitted_quads
            emitted_quads += 1
            tp = tp_pool.tile([P, NCHUNK, QT, P], F16, space=PSUM, tag="tp")
            for j in range(QT):
                t = q * QT + j
                g, a = divmod(t, GT)
                for k in range(NCHUNK):
                    nc.tensor.transpose(
                        tp[:, k, j, :], xg[g][:, a, ts(k, P)], ident_h[:]
                    )
            for k in range(NCHUNK):
                dst = xT[:, k, ds(q * QT * P, QT * P)].rearrange(
                    "p (j u) -> p j u", u=P
                )
                if (q + k) % 2 == 0:
                    nc.vector.tensor_copy(dst, tp[:, k])
                else:
                    nc.scalar.copy(dst, tp[:, k])

        def x_dma(g, j0, j1):
            nc.gpsimd.dma_start(
                xg[g][:, ds(j0, j1 - j0), :],
                x_dram[ds(g * GT * P, GT * P), :].rearrange(
                    "(p j) c -> p j c", j=GT
                )[:, ds(j0, j1 - j0), :],
            )

        for g in range(NG):
            if g == 0:
                for j0, j1 in ((0, 2), (2, 8), (8, GT)):
                    x_dma(g, j0, j1)
                # single activation-table load for the whole kernel
                nc.scalar.add_instruction(
                    mybir.InstLoadActFuncSet(
                        name=nc.get_next_instruction_name(),
                        act_func_set_id=ACT_SET_LN_EXP,
                        ins=[],
                        outs=[],
                    )
                )
                make_identity(nc, identity_f[:])
                nc.vector.tensor_copy(ident_h[:], identity_f[:])
                nc.gpsimd.memset(p8[:], 0.0)
                nc.gpsimd.affine_select(
                    out=p8[:].rearrange("p (b i) -> p b i", i=DH),
                    in_=p8[:].rearrange("p (b i) -> p b i", i=DH),
                    compare_op=ALU.not_equal,
                    fill=1.0,
                    base=0,
                    pattern=[[-1, HEADS], [0, DH]],
                    channel_multiplier=1,
                )
                nc.vector.tensor_copy(p8_r[:], p8[:])
                nc.gpsimd.memset(ones_col[:], 1.0)
                nc.gpsimd.memset(ones_row[:], 1.0)
                nc.vector.tensor_copy(ones_row_h[:], ones_row[:])
                # blockdiag mask factors: -B everywhere (rank 1) + B in-block
                # (rank 4, from the p8 head pattern restricted to one chunk)
                nc.gpsimd.memset(m1024[:], MROW)
                nc.gpsimd.memset(mneg[:], -MCOL)
                nc.vector.tensor_scalar_mul(p8c[:], p8[0 : P // DH, 0:P], MROW)
                nc.vector.tensor_scalar_mul(p8c2[:], p8[0 : P // DH, 0:P], MCOL)
            else:
                x_dma(g, 0, GT)
            if g == 1:
                # weights: casting DMAs (Pool-only) straight to fp16
                for wh, wd in (
                    (wq_h, wq_dram), (wk_h, wk_dram),
                    (wv_h, wv_dram), (wp_h, wp_dram),
                ):
                    nc.gpsimd.dma_start(
                        wh[:], wd.rearrange("(k p) c -> p k c", p=P)
                    )
                nc.sync.dma_start(
                    bp_col[:], bp_dram.rearrange("(k p) -> p k", p=P)
                )
                nc.sync.dma_start(resc_p[:], resc_dram.rearrange("h a b -> h (a b)"))
                nc.vector.tensor_copy(resc_r[:], resc_p[:])

        # PE stream: warmup dummies start the pstate ramp at t~0.5us so the
        # first S matmul already runs at full clock; then S + prep + quads.
        dummies(34)
        for t in range(0, 4):
            s_tile(t, first=(t == 0))
        # prep block 1: Wv transposes (fp16, one packed PSUM bank), rexp row
        tpv4 = prep_pool.tile([P, 4, P], F16, space=PSUM, tag="tpv")
        for q in range(NCHUNK):
            for m in range(NCHUNK):
                nc.tensor.transpose(
                    tpv4[:, 2 * q + m, :], wv_h[:, m, ts(q, P)], ident_h[:]
                )
        for q in range(NCHUNK):
            dst = wvT[:, q, :].rearrange("p (m u) -> p m u", u=P)
            if q == 0:
                nc.vector.tensor_copy(dst, tpv4[:, ds(2 * q, 2), :])
            else:
                nc.scalar.copy(dst, tpv4[:, ds(2 * q, 2), :])
        rexp_ps = prep_pool.tile([P, C], F32, space=PSUM, tag="bc")
        nc.tensor.matmul(rexp_ps[0:1, :], resc_r[:], p8_r[:], start=True, stop=True)
        nc.scalar.copy(rexp_row[:], rexp_ps[0:1, :])
        nc.vector.reciprocal(rexp1i[:], rexp_row[:])
        nc.vector.tensor_mul(rexp2i[:], rexp1i[:], rexp1i[:])
        for t in range(4, 12):
            s_tile(t)
        # prep block 2: rexp^-2 broadcast + scaled Wq (reuses the bc bank)
        r2bc_ps = prep_pool.tile([P, C], F32, space=PSUM, tag="bc")
        nc.tensor.matmul(r2bc_ps[:], ones_row[:], rexp2i[:], start=True, stop=True)
        for k in range(NCHUNK):
            nc.vector.tensor_mul(wq_s[:, k, :], wq_h[:, k, :], r2bc_ps[:])
        for t in range(12, 16):
            s_tile(t)
        prep_stack.close()  # tpv/bc banks free from here
        emit_quad()
        for g in range(1, NG):
            for t in range(g * GT, (g + 1) * GT):
                s_tile(t, last=(t == NT - 1))
            emit_quad()

        # ---------------- phase B: 256x256 attention math (fp16) ----------------
        # S rows chunk0 = [S00|S01]; chunk1 = [S10|S11] with S10 = S01^T
        #   s_row0 = S[0:128, 0:256], s_row1 = S[128:256, 0:256]
        # lhsT for P* chunk (k, m) = S[k-rows, m-cols] = s_row{k}[:, m*128:]
        with tc.tile_pool(name="bsb0", bufs=1) as bsb0:
            s_row0 = bsb0.tile([P, C], F16)
            s_row1 = bsb0.tile([P, C], F16)
            nc.vector.tensor_copy(s_row0[:], s_ps[:, 0:C])
            nc.scalar.copy(s_row1[:, P:C], s_ps[:, C : C + P])
            with tc.tile_pool(name="preb", bufs=1, space=PSUM) as pre_b:
                s10_ps = pre_b.tile([P, P], F16, space=PSUM, tag="bs16")
                nc.tensor.transpose(s10_ps[:], s_row0[:, P:C], ident_h[:])
                nc.vector.tensor_copy(s_row1[:, 0:P], s10_ps[:])
            s_stack.close()  # S bank free from here on

            emit_quad()
            emit_quad()
            dummies(6)

            srows = [s_row0, s_row1]
            bwork_ctx = ExitStack()
            bwork = bwork_ctx.enter_context(
                tc.tile_pool(name="bwork", bufs=4, space=PSUM)
            )
            bsmall = bwork_ctx.enter_context(
                tc.tile_pool(name="bsmall", bufs=1, space=PSUM)
            )
            bsb = bwork_ctx.enter_context(tc.tile_pool(name="bsb", bufs=1))
            # P1 = S Wq, P2 = S Wk
            p1_ps, p2_ps = [], []
            for dst_list, w_h in ((p1_ps, wq_h), (p2_ps, wk_h)):
                for m in range(NCHUNK):
                    pp = bwork.tile(
                        [P, C], F32, space=PSUM,
                        name=f"pps{len(dst_list)}{m}", tag="bw", bufs=4,
                    )
                    for k in range(NCHUNK):
                        nc.tensor.matmul(
                            pp[:], srows[k][:, ts(m, P)], w_h[:, k, :],
                            start=(k == 0), stop=(k == 1),
                        )
                    dst_list.append(pp)

            # evict P1/P2 to fp16; qp for the nq2 fork (reads PSUM directly)
            p1_sb, p2_sb, qpl = [], [], []
            for m in range(NCHUNK):
                psb = bsb.tile([P, C], F16, name=f"p1sb{m}", tag="p1sb", bufs=2)
                if m == 0:
                    nc.vector.tensor_copy(psb[:], p1_ps[m][:])
                else:
                    nc.scalar.copy(psb[:], p1_ps[m][:])
                p1_sb.append(psb)
                qp = bsb.tile([P, C], F16, name=f"qp{m}", tag="qp", bufs=2)
                nc.vector.tensor_mul(qp[:], wq_s[:, m, :], p1_ps[m][:])
                qpl.append(qp)
            for m in range(NCHUNK):
                psb = bsb.tile([P, C], F16, name=f"p2sb{m}", tag="p2sb", bufs=2)
                nc.scalar.copy(psb[:], p2_ps[m][:])
                p2_sb.append(psb)

            emit_quad()
            emit_quad()
            dummies(6)

            # G (block-diag chunks only) with the mask matmuls folded in:
            # out-of-block entries get -MROW*MCOL so they vanish in the exp.
            g_ps = []
            for m in range(NCHUNK):
                gg = bwork.tile([P, P], F32, space=PSUM, name=f"gps{m}", tag="bw", bufs=4)
                for k in range(NCHUNK):
                    nc.tensor.matmul(
                        gg[:], wk_h[:, k, ts(m, P)], p1_sb[k][:, ts(m, P)],
                        start=(k == 0), stop=False,
                    )
                nc.tensor.matmul(gg[:], m1024[:], mneg[:], start=False, stop=False)
                nc.tensor.matmul(gg[:], p8c[:], p8c2[:], start=False, stop=True)
                g_ps.append(gg)

            # nq2 fork: colsum(qp) -> rq' = rsqrt(nq2 * rexp^-2) = rq * rescale
            nq2_ps = bsmall.tile([1, C], F32, space=PSUM, tag="bs")
            for k in range(NCHUNK):
                nc.tensor.matmul(
                    nq2_ps[:], ones_col[:], qpl[k][:], start=(k == 0), stop=(k == 1)
                )
            lnq = bsb.tile([1, C], F32)
            nc.scalar.activation(lnq[:], nq2_ps[:], ACTF.Ln)
            rq_h = bsb.tile([1, C], F16)
            nc.scalar.activation(rq_h[:], lnq[:], ACTF.Exp, scale=-0.5)
            csbc_ps = bsmall.tile([P, C], F32, space=PSUM, tag="bs")
            nc.tensor.matmul(csbc_ps[:], ones_row_h[:], rq_h[:], start=True, stop=True)
            csbc_sb = bsb.tile([P, C], F16)
            nc.vector.tensor_copy(csbc_sb[:], csbc_ps[:])

            # nk2 fork: diag(Wk^T P2) via Kgram + identity-masked row-reduce
            nk2 = bsb.tile([P, NCHUNK], F32)
            scraps = [bsb.tile([P, P], F32, name=f"scrap{m}") for m in range(NCHUNK)]
            for m in range(NCHUNK):
                kg = bwork.tile([P, P], F32, space=PSUM, name=f"kg{m}", tag="bw", bufs=4)
                for k in range(NCHUNK):
                    nc.tensor.matmul(
                        kg[:], wk_h[:, k, ts(m, P)], p2_sb[k][:, ts(m, P)],
                        start=(k == 0), stop=(k == 1),
                    )
                nc.vector.scalar_tensor_tensor(
                    out=scraps[m][:],
                    in0=kg[:],
                    scalar=1.0,
                    in1=identity_f[:],
                    op0=ALU.mult,
                    op1=ALU.mult,
                    accum_out=nk2[:, m : m + 1],
                )
            lnk = bsb.tile([P, NCHUNK], F32)
            nc.scalar.activation(lnk[:], nk2[:], ACTF.Ln)
            rk = bsb.tile([P, NCHUNK], F32)
            nc.scalar.activation(rk[:], lnk[:], ACTF.Exp, scale=-0.5)

            emit_quad()
            emit_quad()
            dummies(8)

            # softmax tail + T1 + Wbig
            t1_sb = []
            for m in range(NCHUNK):
                dummies(3)
                tt = bsb.tile([P, P], F16, name=f"t{m}", tag="t", bufs=2)
                nc.vector.tensor_mul(tt[:], g_ps[m][:], csbc_sb[:, ts(m, P)])
                e = bsb.tile([P, P], F16, name=f"e{m}", tag="e", bufs=2)
                den = bsb.tile([P, 1], F32, name=f"den{m}", tag="den", bufs=2)
                nc.scalar.activation(
                    e[:], tt[:], ACTF.Exp, scale=rk[:, m : m + 1], accum_out=den[:]
                )
                rden = bsb.tile([P, 1], F32, name=f"rden{m}", tag="rden", bufs=2)
                nc.vector.reciprocal(rden[:], den[:])
                a_m = bsb.tile([P, P], F16, name=f"a{m}", tag="a", bufs=2)
                nc.vector.tensor_scalar_mul(a_m[:], e[:], rden[:])
                t1p = bwork.tile(
                    [P, C], F32, space=PSUM, name=f"t1ps{m}", tag="bw", bufs=4
                )
                nc.tensor.matmul(t1p[:], a_m[:], wp_h[:, m, :], start=True, stop=True)
                t1s = bsb.tile([P, C], F16, name=f"t1sb{m}", tag="t1sb", bufs=2)
                if m == 0:
                    nc.vector.tensor_copy(t1s[:], t1p[:])
                else:
                    nc.scalar.copy(t1s[:], t1p[:])
                t1_sb.append(t1s)

            for m in range(NCHUNK):
                wbp = bwork.tile(
                    [P, C], F32, space=PSUM, name=f"wbps{m}", tag="bw", bufs=4
                )
                for q in range(NCHUNK):
                    nc.tensor.matmul(
                        wbp[:], wvT[:, q, ts(m, P)], t1_sb[q][:],
                        start=(q == 0), stop=(q == 1),
                    )
                if m == 0:
                    nc.vector.tensor_copy(wbig[m][:], wbp[:])
                else:
                    nc.scalar.copy(wbig[m][:], wbp[:])
                dummies(2)
            bwork_ctx.close()

        # ------- pass 2: out^T = Wbig^T xT + bp, 32 pipelined fp16 DMAs -------
        # Output quad oq covers TRUE tokens [oq*512, (oq+1)*512): group
        # g = oq//4, u in [32*(oq%4), +32), all j in [0,16).  The matmul rhs
        # gathers the scattered xT positions; the eviction AP un-permutes
        # (j,u) -> 16u+j so each DMA writes a contiguous token range.
        with tc.tile_pool(name="ops", bufs=5, space=PSUM) as ops, tc.tile_pool(
            name="outb", bufs=4
        ) as outb:
            for oq in range(NQ):
                g, uq = divmod(oq, NQ // NG)
                # keep transposes one group ahead of the output quads
                while emitted_quads < min((g + 2) * (NQ // NG), NQ):
                    emit_quad()
                ob = outb.tile([P, NCHUNK, QT * P], F16, tag="ob")
                for m in range(NCHUNK):
                    o_ps = ops.tile([P, QT * P], F32, space=PSUM, tag="o")
                    for k in range(NCHUNK):
                        rhs = xT[:, k, ds(g * GT * P, GT * P)].rearrange(
                            "p (j u) -> p j u", u=P
                        )[:, :, ds(uq * 32, 32)]
                        nc.tensor.matmul(
                            o_ps[:].rearrange("p (j u) -> p j u", u=32),
                            wbig[k][:, ts(m, P)],
                            rhs,
                            start=(k == 0),
                            stop=(k == 1),
                        )
                    # evict + bias; o_ps columns are (j, u), true token
                    # offset within the quad is 16u + j
                    dst = ob[:, m, :].rearrange("p (u j) -> p u j", j=GT)
                    src = o_ps[:].rearrange("p (j u) -> p u j", u=32)
                    if m == 0:
                        nc.vector.tensor_scalar_add(dst, src, bp_col[:, m : m + 1])
                    else:
                        nc.scalar.activation(
                            dst, src, ACTF.Identity, bias=bp_col[:, m : m + 1]
                        )
                nc.sync.dma_start(outT_v[:, :, ds(oq * QT * P, QT * P)], ob[:])

        tp_stack.close()

    return nc


_NC_CACHE = None


def _get_nc():
    global _NC_CACHE
    if _NC_CACHE is None:
        nc = bacc.Bacc(
            "TRN2",
            target_bir_lowering=False,
            debug=False,
            enable_asserts=False,
            num_devices=N_CORES,
        )
        _build_kernel(nc)
        nc.compile()
        nc.m = get_hw_module(nc.m)
        _NC_CACHE = nc
    return _NC_CACHE


def _make_in_maps(x_in, Wq, Wk, Wv, rescale, Wp, bp):
    x_in = np.ascontiguousarray(np.asarray(x_in, dtype=np.float32))
    maps = []
    for core in range(N_CORES):
        maps.append(
            {
                "x_in": x_in[core].reshape(N, C),
                "Wq": np.asarray(Wq, np.float32),
                "Wk": np.asarray(Wk, np.float32),
                "Wv": np.asarray(Wv, np.float32),
                "rescale": np.asarray(rescale, np.float32),
                "Wp": np.asarray(Wp, np.float32),
                "bp": np.asarray(bp, np.float32),
            }
        )
    return maps


def run_on_hw(inputs: dict, trace: bool = False, tmpdir: str | None = None):
    """Returns (full_output [8,128,128,256] f32, BassKernelResults)."""
    nc = _get_nc()
    in_maps = _make_in_maps(**inputs)
    res = bass_utils.run_bass_kernel_spmd(
        nc, in_maps, core_ids=list(range(N_CORES)), trace=trace, tmpdir=tmpdir
    )
    out = np.stack(
        [
            np.asarray(res.results[c]["out"], dtype=np.float32).T.reshape(H, W, C)
            for c in range(N_CORES)
        ]
    )
    return out, res


def kernel(x_in, Wq, Wk, Wv, rescale, Wp, bp) -> np.ndarray:
    out, _ = run_on_hw(
        dict(x_in=x_in, Wq=Wq, Wk=Wk, Wv=Wv, rescale=rescale, Wp=Wp, bp=bp)
    )
    return out
